# revision 17
# baseline (speedup 1.0000x reference)
"""Trainium2 Bass kernel for the GNN message-passing module.

Reference computation (per batch b):
    msg_n = node @ Wn + bn                      (N, MID)
    msg_h = hidden @ Wh + bh                    (N, MID)
    msg_e = edge @ We + be                      (N, N, MID)
    msg_g = graph @ Wg + bg                     (MID,)
    msgs[i,j,:] = msg_n[j] + msg_h[i] + msg_e[i,j] + msg_g
    out_msgs[j,:] = max_i(msgs[i,j,:] * adj[i,j])
    ret = node @ Wo1 + bo1 + hidden @ Wo2 + bo2 + out_msgs @ Wo3 + bo3

Kernel strategy (data-parallel, one batch per core across 8 cores):
  - Orientation: channels on SBUF partitions, j (receiver) on the free dim.
  - The multiplicative {0,1} adjacency mask is converted to an additive mask
    adjm = (adj-1)*1e30 in {0, -1e30}, folded into the PE accumulation as a
    rank-1 matmul (ones_c (x) adjm_row_i).  A per-j correction vector cvec
    restores the exact max semantics (masked entries contribute 0 to the max,
    all-kept columns must not see the 0 candidate).
  - msg_n is constant in i, so it is pulled out of the max and added once.
  - h_i = msg_h[i] + msg_g + (bn+bh+be+bg) enters through the fused DVE op
    acc = max(acc, psum_i + h_col_i) (scalar_tensor_tensor, one op per i).
  - fp32 data is fed to the PE as float32r (replicated fp32), which streams at
    1 cycle/row for free dims >= 256 while keeping full fp32 precision.
"""

from contextlib import ExitStack

import numpy as np

B, N, D, E, G, MID, OUT = 8, 256, 128, 128, 128, 128, 128
NCORES = 8
BIG = 1.0e30
GI = 8  # edge rows (i values) per DMA group
CH = 32  # adjm rows per staging chunk
NT = N // 128  # number of 128-row tiles along N

_WNAMES = ["Wn", "Wh", "We", "Wg", "Wo1", "Wo2", "Wo3"]
_BNAMES = ["bn", "bh", "be", "bg", "bo1", "bo2", "bo3"]

_CACHE = {}


def _ensure_path():
    try:
        import concourse.bass  # noqa: F401
    except ImportError:
        import sys

        for p in ("/opt/trn_rl_repo", "/root/.axon_site/_ro/trn_rl_repo"):
            if p not in sys.path:
                sys.path.insert(0, p)
        import concourse.bass  # noqa: F401


def _kernel_body(ctx, tc, aps, rep=0):
    import concourse.bass as bass  # noqa: F401
    from concourse import masks, mybir

    nc = tc.nc
    f32 = mybir.dt.float32
    f32r = mybir.dt.float32r
    Alu = mybir.AluOpType

    edge = aps["edge"]
    node = aps["node"]
    hidden = aps["hidden"]
    graph = aps["graph"]
    adj = aps["adj"]
    out = aps["out"]

    const = ctx.enter_context(tc.tile_pool(name="const", bufs=1))
    ps_pool = ctx.enter_context(tc.tile_pool(name="ps", bufs=1, space="PSUM"))
    trpool = ctx.enter_context(tc.tile_pool(name="trp", bufs=3, space="PSUM"))
    opool = ctx.enter_context(tc.tile_pool(name="op", bufs=4, space="PSUM"))
    epool = ctx.enter_context(tc.tile_pool(name="edgein", bufs=3))
    etpool = ctx.enter_context(tc.tile_pool(name="edgeT", bufs=10))
    arpool = ctx.enter_context(tc.tile_pool(name="adjrow", bufs=2))

    # ---- constants -------------------------------------------------------
    ident = const.tile([128, 128], f32)
    masks.make_identity(nc, ident[:])
    ident_r = const.tile([128, 128], f32r)
    nc.vector.tensor_copy(ident_r[:], ident[:])

    ones_f = const.tile([1, 256], f32)
    nc.vector.memset(ones_f[:], 1.0)
    ones_row = const.tile([1, 256], f32r)
    nc.vector.tensor_copy(ones_row[:], ones_f[:])
    ones_1c = const.tile([1, 128], f32r)
    nc.vector.tensor_copy(ones_1c[:], ones_f[:, 0:128])
    ones_11 = const.tile([1, 1], f32r)
    nc.vector.tensor_copy(ones_11[:], ones_f[:, 0:1])
    ones_colf = const.tile([128, 1], f32)
    nc.vector.memset(ones_colf[:], 1.0)
    ones_col = const.tile([128, 1], f32r)
    nc.vector.tensor_copy(ones_col[:], ones_colf[:])

    W_sb = {}
    for w in _WNAMES:
        Wf = const.tile([128, 128], f32, name=f"r{rep}_Wf_{w}", tag=f"Wf_{w}")
        nc.sync.dma_start(Wf[:], aps[w])
        W_sb[w] = const.tile([128, 128], f32r, name=f"r{rep}_W_{w}", tag=f"W_{w}")
        nc.vector.tensor_copy(W_sb[w][:], Wf[:])
    B_sb = {}
    for b in _BNAMES:
        Bf = const.tile([1, 128], f32, name=f"r{rep}_Bf_{b}", tag=f"Bf_{b}")
        nc.sync.dma_start(Bf[:], aps[b].rearrange("(o k) -> o k", o=1))
        B_sb[b] = const.tile([1, 128], f32r, name=f"r{rep}_B_{b}", tag=f"B_{b}")
        nc.vector.tensor_copy(B_sb[b][:], Bf[:])

    graph_colf = const.tile([128, 1], f32)
    nc.sync.dma_start(graph_colf[:], graph.rearrange("(p o) -> p o", o=1))
    graph_col = const.tile([128, 1], f32r)
    nc.vector.tensor_copy(graph_col[:], graph_colf[:])

    node_nat = const.tile([128, NT * 128], f32)
    nc.sync.dma_start(
        node_nat[:].rearrange("p (t d) -> p t d", t=NT),
        node.rearrange("(t p) d -> p t d", p=128),
    )
    hid_nat = const.tile([128, NT * 128], f32)
    nc.sync.dma_start(
        hid_nat[:].rearrange("p (t d) -> p t d", t=NT),
        hidden.rearrange("(t p) d -> p t d", p=128),
    )
    adj_nat = const.tile([128, NT * 256], mybir.dt.int32)
    nc.sync.dma_start(
        adj_nat[:].rearrange("p (t j) -> p t j", t=NT),
        adj.rearrange("(t p) j -> p t j", p=128),
    )

    # ---- per-batch precompute -------------------------------------------
    # nodeT / hidT: (d, n) layouts via PE transpose
    nodeT = const.tile([128, 256], f32r)
    hidT = const.tile([128, 256], f32r)
    for nat, T in ((node_nat, nodeT), (hid_nat, hidT)):
        ps = ps_pool.tile([128, 256], f32, tag="ps")
        for t in range(NT):
            nc.tensor.transpose(
                ps[:, t * 128 : (t + 1) * 128],
                nat[:, t * 128 : (t + 1) * 128],
                ident[:],
            )
        nc.scalar.copy(T[:], ps[:])

    # r0 = graph @ Wg + (bn + bh + be + bg), a (1, MID) row
    ps_r0 = ps_pool.tile([128, 256], f32, tag="ps")
    nc.tensor.matmul(
        ps_r0[0:1, 0:128],
        graph_col[:],
        W_sb["Wg"][:],
        start=True,
        stop=False,
    )
    for k, bname in enumerate(["bn", "bh", "be", "bg"]):
        nc.tensor.matmul(
            ps_r0[0:1, 0:128],
            ones_11[:],
            B_sb[bname][:],
            start=False,
            stop=(k == 3),
        )
    r0 = const.tile([1, 128], f32r)
    nc.scalar.copy(r0[:], ps_r0[0:1, 0:128])

    # H_T[c, i] = (hidden @ Wh).T + r0[c]  (h_i rows, channel-major)
    ps_h = ps_pool.tile([128, 256], f32, tag="ps")
    nc.tensor.matmul(
        ps_h[:], W_sb["Wh"][:], hidT[:],
        start=True, stop=False,
    )
    nc.tensor.matmul(
        ps_h[:], r0[:], ones_row[:],
        start=False, stop=True,
    )
    H_T = const.tile([128, 256], f32)
    nc.scalar.copy(H_T[:], ps_h[:])

    # msg_nT[c, j] = (node @ Wn).T  (no bias: biases live in r0)
    ps_mn = ps_pool.tile([128, 256], f32, tag="ps")
    nc.tensor.matmul(
        ps_mn[:], W_sb["Wn"][:], nodeT[:],
        start=True, stop=True,
    )
    msg_nT = const.tile([128, 256], f32)
    nc.scalar.copy(msg_nT[:], ps_mn[:])

    # adjm = (adj - 1) * BIG  in {0, -BIG}, natural (i%128, (i//128, j)) layout
    adj_f = const.tile([128, NT * 256], f32)
    nc.vector.tensor_copy(adj_f[:], adj_nat[:])
    adjm = const.tile([128, NT * 256], f32r)
    nc.vector.tensor_scalar(adjm[:], adj_f[:], -1.0, BIG, Alu.add, Alu.mult)

    # cvec[j] = -BIG - max(sum_i adjm[i,j], -BIG)  -> -BIG if column fully
    # kept (k=256), else 0 (the "0 candidate" of the reference max)
    ps_s = ps_pool.tile([128, 256], f32, tag="ps")
    for t in range(NT):
        nc.tensor.matmul(
            ps_s[0:1, :],
            ones_col[:],
            adjm[:, t * 256 : (t + 1) * 256],
            start=(t == 0),
            stop=(t == NT - 1),
        )
    # cvec = -BIG if column fully kept (s == 0), else 0  (threshold form is
    # robust to f32r rounding of the BIG constants)
    cvec = const.tile([1, 256], f32r)
    nc.vector.tensor_scalar(cvec[:], ps_s[0:1, :], -1.0e29, -BIG, Alu.is_ge, Alu.mult)

    # running max accumulators (channels x j), round-robin x4 so consecutive
    # DVE ops never self-wait on the previous accumulation
    NACC = 8
    accs = []
    for q in range(NACC):
        a_ = const.tile([128, 256], f32, name=f"r{rep}_acc{q}", tag=f"acc{q}")
        nc.vector.memset(a_[:], -3.0e38)
        accs.append(a_)

    # ---- main loop over sender rows i -----------------------------------
    # Software-pipelined: group g's transposes+copies are emitted BEFORE
    # group g-1's matmuls+max ops, so the PE never stalls in-order on the
    # PE -> ACT(copy) -> PE round trip within one i.
    edge_r = edge.rearrange("i (t p) e -> p i t e", p=128)
    NG = N // GI

    def stage_a(g):
        """Load + transpose + evacuate group g; returns (128,512) eT pairs."""
        i0 = g * GI
        et = epool.tile([128, GI * NT * 128], f32r, tag="et", name=f"r{rep}_et{g}")
        # alternate the two DMA issue rings so neither sequencer serializes
        dma_eng = nc.sync if g % 2 == 0 else nc.gpsimd
        dma_eng.dma_start(
            et[:].rearrange("p (a t e) -> p a t e", a=GI, t=NT),
            edge_r[:, i0 : i0 + GI, :, :].bitcast(f32r),
        )
        eTs = []
        for p in range(GI // 2):
            # one PSUM bank holds the transposes of two i values
            trp = trpool.tile([128, 512], f32r, tag="trp", name=f"r{rep}_trp{g}_{p}")
            for u in range(2):
                a = 2 * p + u
                for t in range(NT):
                    nc.tensor.transpose(
                        trp[:, (u * NT + t) * 128 : (u * NT + t + 1) * 128],
                        et[:, (a * NT + t) * 128 : (a * NT + t + 1) * 128],
                        ident_r[:],
                    )
            eT = etpool.tile([128, 512], f32r, tag="eT", name=f"r{rep}_eT{g}_{p}")
            nc.scalar.copy(eT[:], trp[:].bitcast(f32))
            eTs.append(eT)
        return eTs

    def stage_b(g, eTs, ar_chunk):
        """Matmuls + masked-max accumulation for group g (i pairs)."""
        i0 = g * GI
        for p in range(GI // 2):
            i = i0 + 2 * p
            op = opool.tile([128, 512], f32, tag="op", name=f"r{rep}_op{g}_{p}")
            nc.tensor.matmul(
                op[:], W_sb["We"][:], eTs[p][:],
                start=True, stop=False,
            )
            nc.tensor.matmul(
                op[:],
                ones_1c[:],
                ar_chunk[0:1, (i % CH) * 256 : (i % CH + 2) * 256],
                start=False,
                stop=True,
            )
            for u in range(2):
                iu = i + u
                a_ = accs[iu % NACC]
                nc.vector.scalar_tensor_tensor(
                    a_[:],
                    op[:, u * 256 : (u + 1) * 256],
                    H_T[:, iu : iu + 1],
                    a_[:],
                    Alu.add,
                    Alu.max,
                )

    def ar_stage(i0):
        """Stage adjm rows i0..i0+CH-1 at partition 0 (f32r, ACT HWDGE ring)."""
        ar = arpool.tile([1, CH * 256], f32r, tag="ar", name=f"r{rep}_ar{i0}")
        t_i = i0 // 128
        p0 = i0 % 128
        nc.scalar.dma_start(
            ar[:].rearrange("o (a j) -> o a j", a=CH),
            adjm[p0 : p0 + CH, t_i * 256 : (t_i + 1) * 256],
        )
        return ar

    GPC = CH // GI       # groups per adjm chunk
    prev = None          # (g, eTs)
    chunks = {0: ar_stage(0)}
    ar_chunk = chunks[0]
    for g in range(NG):
        # prefetch the next chunk halfway through the current one
        ck = (g * GI) // CH
        if g % GPC == GPC // 2 and (ck + 1) * CH < N:
            chunks[ck + 1] = ar_stage((ck + 1) * CH)
        eTs = stage_a(g)
        if prev is not None:
            stage_b(prev[0], prev[1], chunks[(prev[0] * GI) // CH])
        prev = (g, eTs)
    stage_b(prev[0], prev[1], chunks[(prev[0] * GI) // CH])

    # ---- finalize --------------------------------------------------------
    mrg = []
    for q in range(4):
        m_ = const.tile([128, 256], f32, name=f"r{rep}_mrg{q}", tag=f"mrg{q}")
        nc.vector.tensor_tensor(m_[:], accs[2 * q][:], accs[2 * q + 1][:], Alu.max)
        mrg.append(m_)
    a01 = const.tile([128, 256], f32)
    nc.vector.tensor_tensor(a01[:], mrg[0][:], mrg[1][:], Alu.max)
    a23 = const.tile([128, 256], f32)
    nc.vector.tensor_tensor(a23[:], mrg[2][:], mrg[3][:], Alu.max)
    acc = const.tile([128, 256], f32)
    nc.vector.tensor_tensor(acc[:], a01[:], a23[:], Alu.max)

    ps_cv = ps_pool.tile([128, 256], f32, tag="ps")
    nc.tensor.matmul(
        ps_cv[:], ones_1c[:], cvec[:],
        start=True, stop=True,
    )
    msgsT = const.tile([128, 256], f32)
    nc.vector.tensor_tensor(msgsT[:], acc[:], msg_nT[:], Alu.add)
    resT = const.tile([128, 256], f32r)
    nc.vector.tensor_tensor(resT[:], msgsT[:], ps_cv[:], Alu.max)

    # ret_T (o, n)
    ps_ret = ps_pool.tile([128, 256], f32, tag="ps")
    nc.tensor.matmul(
        ps_ret[:], W_sb["Wo1"][:], nodeT[:],
        start=True, stop=False,
    )
    nc.tensor.matmul(
        ps_ret[:], W_sb["Wo2"][:], hidT[:],
        start=False, stop=False,
    )
    nc.tensor.matmul(
        ps_ret[:], W_sb["Wo3"][:], resT[:],
        start=False, stop=False,
    )
    for k, bname in enumerate(["bo1", "bo2", "bo3"]):
        nc.tensor.matmul(
            ps_ret[:],
            B_sb[bname][:],
            ones_row[:],
            start=False,
            stop=(k == 2),
        )
    retT = const.tile([128, 256], f32)
    nc.scalar.copy(retT[:], ps_ret[:])

    ps_out = ps_pool.tile([128, 256], f32, tag="ps")
    for t in range(NT):
        nc.tensor.transpose(
            ps_out[:, t * 128 : (t + 1) * 128],
            retT[:, t * 128 : (t + 1) * 128],
            ident[:],
        )
    out_sb = const.tile([128, 256], f32)
    nc.scalar.copy(out_sb[:], ps_out[:])
    nc.sync.dma_start(
        out.rearrange("(t p) o -> p t o", p=128),
        out_sb[:].rearrange("p (t o) -> p t o", t=NT),
    )


def build_nc(repeat=1):
    """Build the (single-core SPMD) Bass program; returns nc."""
    _ensure_path()
    import concourse.tile as tile
    from concourse import bacc, mybir

    f32 = mybir.dt.float32
    i32 = mybir.dt.int32

    nc = bacc.Bacc(
        "TRN2", target_bir_lowering=False, debug=False, num_devices=NCORES
    )
    aps = {
        "edge": nc.dram_tensor("edge", [N, N, E], f32, kind="ExternalInput").ap(),
        "node": nc.dram_tensor("node", [N, D], f32, kind="ExternalInput").ap(),
        "hidden": nc.dram_tensor("hidden", [N, D], f32, kind="ExternalInput").ap(),
        "graph": nc.dram_tensor("graph", [G], f32, kind="ExternalInput").ap(),
        "adj": nc.dram_tensor("adj", [N, N], i32, kind="ExternalInput").ap(),
        "out": nc.dram_tensor("out", [N, OUT], f32, kind="ExternalOutput").ap(),
    }
    for w in _WNAMES:
        aps[w] = nc.dram_tensor(w, [128, 128], f32, kind="ExternalInput").ap()
    for b in _BNAMES:
        aps[b] = nc.dram_tensor(b, [128], f32, kind="ExternalInput").ap()

    with tile.TileContext(nc) as tc:
        for rep in range(repeat):
            with ExitStack() as ctx:
                _kernel_body(ctx, tc, aps, rep=rep)
    nc.compile()
    return nc


def _get_nc():
    if "nc" not in _CACHE:
        _CACHE["nc"] = build_nc()
    return _CACHE["nc"]


def make_in_maps(**inputs):
    """Shard full inputs into per-core input maps (batch-parallel)."""
    in_maps = []
    for c in range(NCORES):
        m = {
            "edge": np.ascontiguousarray(inputs["edge_fts"][c], np.float32),
            "node": np.ascontiguousarray(inputs["node_fts"][c], np.float32),
            "hidden": np.ascontiguousarray(inputs["hidden"][c], np.float32),
            "graph": np.ascontiguousarray(inputs["graph_fts"][c], np.float32),
            "adj": np.ascontiguousarray(inputs["adj_mat"][c], np.int32),
        }
        for w in _WNAMES:
            m[w] = np.ascontiguousarray(inputs[w], np.float32)
        for b in _BNAMES:
            m[b] = np.ascontiguousarray(inputs[b], np.float32)
        in_maps.append(m)
    return in_maps


def kernel(**inputs) -> np.ndarray:
    """Full-input entry point: shards over 8 cores, returns (B, N, OUT)."""
    _ensure_path()
    from concourse import bass_utils

    nc = _get_nc()
    in_maps = make_in_maps(**inputs)
    res = bass_utils.run_bass_kernel_spmd(nc, in_maps, core_ids=list(range(NCORES)))
    outs = [res.results[c]["out"] for c in range(NCORES)]
    return np.stack(outs, axis=0).astype(np.float32)


def kernel_traced(tmpdir=None, **inputs):
    """Like kernel(), but requests an NTFF profile; returns (out, results)."""
    _ensure_path()
    from concourse import bass_utils

    nc = _get_nc()
    in_maps = make_in_maps(**inputs)
    res = bass_utils.run_bass_kernel_spmd(
        nc, in_maps, core_ids=list(range(NCORES)), trace=True, tmpdir=tmpdir
    )
    outs = [res.results[c]["out"] for c in range(NCORES)]
    return np.stack(outs, axis=0).astype(np.float32), res


if __name__ == "__main__":
    rng = np.random.default_rng(0)
    inputs = {
        "node_fts": rng.normal(size=(B, N, D)).astype(np.float32),
        "edge_fts": rng.normal(size=(B, N, N, E)).astype(np.float32),
        "graph_fts": rng.normal(size=(B, G)).astype(np.float32),
        "adj_mat": rng.integers(0, 2, size=(B, N, N)).astype(np.int32),
        "hidden": rng.normal(size=(B, N, D)).astype(np.float32),
    }
    s = 0.02
    for w in _WNAMES:
        inputs[w] = (s * rng.normal(size=(128, 128))).astype(np.float32)
    for b in _BNAMES:
        inputs[b] = np.zeros(128, np.float32)
    out = kernel(**inputs)
    print(out.shape, out.dtype)


# revision 19
# speedup vs baseline: 1.7674x; 1.7674x over previous
"""Trainium2 Bass kernel for the GNN message-passing module.

Reference computation (per batch b):
    msg_n = node @ Wn + bn                      (N, MID)
    msg_h = hidden @ Wh + bh                    (N, MID)
    msg_e = edge @ We + be                      (N, N, MID)
    msg_g = graph @ Wg + bg                     (MID,)
    msgs[i,j,:] = msg_n[j] + msg_h[i] + msg_e[i,j] + msg_g
    out_msgs[j,:] = max_i(msgs[i,j,:] * adj[i,j])
    ret = node @ Wo1 + bo1 + hidden @ Wo2 + bo2 + out_msgs @ Wo3 + bo3

Kernel strategy (data-parallel, one batch per core across 8 cores):
  - Orientation: channels on SBUF partitions, j (receiver) on the free dim.
  - The multiplicative {0,1} adjacency mask is converted to an additive mask
    adjm = (adj-1)*1e30 in {0, -1e30}, folded into the PE accumulation as a
    rank-1 matmul (ones_c (x) adjm_row_i).  A per-j correction vector cvec
    restores the exact max semantics (masked entries contribute 0 to the max,
    all-kept columns must not see the 0 candidate).
  - msg_n is constant in i, so it is pulled out of the max and added once.
  - h_i = msg_h[i] + msg_g + (bn+bh+be+bg) enters through the fused DVE op
    acc = max(acc, psum_i + h_col_i) (scalar_tensor_tensor, one op per i).
  - fp32 data is fed to the PE as float32r (replicated fp32), which streams at
    1 cycle/row for free dims >= 256 while keeping full fp32 precision.
"""

from contextlib import ExitStack

import numpy as np

B, N, D, E, G, MID, OUT = 8, 256, 128, 128, 128, 128, 128
NCORES = 8
BIG = 1.0e30
GI = 8  # edge rows (i values) per DMA group
CH = 32  # adjm rows per staging chunk
NT = N // 128  # number of 128-row tiles along N

_WNAMES = ["Wn", "Wh", "We", "Wg", "Wo1", "Wo2", "Wo3"]
_BNAMES = ["bn", "bh", "be", "bg", "bo1", "bo2", "bo3"]

_CACHE = {}


def _ensure_path():
    try:
        import concourse.bass  # noqa: F401
    except ImportError:
        import sys

        for p in ("/opt/trn_rl_repo", "/root/.axon_site/_ro/trn_rl_repo"):
            if p not in sys.path:
                sys.path.insert(0, p)
        import concourse.bass  # noqa: F401


def _kernel_body(ctx, tc, aps, rep=0, edge_groups=None):
    import concourse.bass as bass  # noqa: F401
    from concourse import masks, mybir

    nc = tc.nc
    f32 = mybir.dt.float32
    f32r = mybir.dt.float32r
    Alu = mybir.AluOpType

    edge = aps["edge"]
    node = aps["node"]
    hidden = aps["hidden"]
    graph = aps["graph"]
    adj = aps["adj"]
    out = aps["out"]

    const = ctx.enter_context(tc.tile_pool(name="const", bufs=1))
    ps_pool = ctx.enter_context(tc.tile_pool(name="ps", bufs=1, space="PSUM"))
    trpool = ctx.enter_context(tc.tile_pool(name="trp", bufs=3, space="PSUM"))
    opool = ctx.enter_context(tc.tile_pool(name="op", bufs=4, space="PSUM"))
    epool = ctx.enter_context(tc.tile_pool(name="edgein", bufs=3))
    etpool = ctx.enter_context(tc.tile_pool(name="edgeT", bufs=10))
    arpool = ctx.enter_context(tc.tile_pool(name="adjrow", bufs=2))

    # ---- constants -------------------------------------------------------
    ident = const.tile([128, 128], f32)
    masks.make_identity(nc, ident[:])
    ident_r = const.tile([128, 128], f32r)
    nc.vector.tensor_copy(ident_r[:], ident[:])

    ones_f = const.tile([1, 256], f32)
    nc.vector.memset(ones_f[:], 1.0)
    ones_row = const.tile([1, 256], f32r)
    nc.vector.tensor_copy(ones_row[:], ones_f[:])
    ones_1c = const.tile([1, 128], f32r)
    nc.vector.tensor_copy(ones_1c[:], ones_f[:, 0:128])
    ones_11 = const.tile([1, 1], f32r)
    nc.vector.tensor_copy(ones_11[:], ones_f[:, 0:1])
    ones_colf = const.tile([128, 1], f32)
    nc.vector.memset(ones_colf[:], 1.0)
    ones_col = const.tile([128, 1], f32r)
    nc.vector.tensor_copy(ones_col[:], ones_colf[:])

    W_sb = {}
    for w in _WNAMES:
        Wf = const.tile([128, 128], f32, name=f"r{rep}_Wf_{w}", tag=f"Wf_{w}")
        nc.sync.dma_start(Wf[:], aps[w])
        W_sb[w] = const.tile([128, 128], f32r, name=f"r{rep}_W_{w}", tag=f"W_{w}")
        nc.vector.tensor_copy(W_sb[w][:], Wf[:])
    B_sb = {}
    for b in _BNAMES:
        Bf = const.tile([1, 128], f32, name=f"r{rep}_Bf_{b}", tag=f"Bf_{b}")
        nc.sync.dma_start(Bf[:], aps[b].rearrange("(o k) -> o k", o=1))
        B_sb[b] = const.tile([1, 128], f32r, name=f"r{rep}_B_{b}", tag=f"B_{b}")
        nc.vector.tensor_copy(B_sb[b][:], Bf[:])

    graph_colf = const.tile([128, 1], f32)
    nc.sync.dma_start(graph_colf[:], graph.rearrange("(p o) -> p o", o=1))
    graph_col = const.tile([128, 1], f32r)
    nc.vector.tensor_copy(graph_col[:], graph_colf[:])

    node_nat = const.tile([128, NT * 128], f32)
    nc.sync.dma_start(
        node_nat[:].rearrange("p (t d) -> p t d", t=NT),
        node.rearrange("(t p) d -> p t d", p=128),
    )
    hid_nat = const.tile([128, NT * 128], f32)
    nc.sync.dma_start(
        hid_nat[:].rearrange("p (t d) -> p t d", t=NT),
        hidden.rearrange("(t p) d -> p t d", p=128),
    )
    adj_nat = const.tile([128, NT * 256], mybir.dt.int32)
    nc.sync.dma_start(
        adj_nat[:].rearrange("p (t j) -> p t j", t=NT),
        adj.rearrange("(t p) j -> p t j", p=128),
    )

    # ---- per-batch precompute -------------------------------------------
    # nodeT / hidT: (d, n) layouts via PE transpose
    nodeT = const.tile([128, 256], f32r)
    hidT = const.tile([128, 256], f32r)
    for nat, T in ((node_nat, nodeT), (hid_nat, hidT)):
        ps = ps_pool.tile([128, 256], f32, tag="ps")
        for t in range(NT):
            nc.tensor.transpose(
                ps[:, t * 128 : (t + 1) * 128],
                nat[:, t * 128 : (t + 1) * 128],
                ident[:],
            )
        nc.scalar.copy(T[:], ps[:])

    # r0 = graph @ Wg + (bn + bh + be + bg), a (1, MID) row
    ps_r0 = ps_pool.tile([128, 256], f32, tag="ps")
    nc.tensor.matmul(
        ps_r0[0:1, 0:128],
        graph_col[:],
        W_sb["Wg"][:],
        start=True,
        stop=False,
    )
    for k, bname in enumerate(["bn", "bh", "be", "bg"]):
        nc.tensor.matmul(
            ps_r0[0:1, 0:128],
            ones_11[:],
            B_sb[bname][:],
            start=False,
            stop=(k == 3),
        )
    r0 = const.tile([1, 128], f32r)
    nc.scalar.copy(r0[:], ps_r0[0:1, 0:128])

    # H_T[c, i] = (hidden @ Wh).T + r0[c]  (h_i rows, channel-major)
    ps_h = ps_pool.tile([128, 256], f32, tag="ps")
    nc.tensor.matmul(
        ps_h[:], W_sb["Wh"][:], hidT[:],
        start=True, stop=False,
    )
    nc.tensor.matmul(
        ps_h[:], r0[:], ones_row[:],
        start=False, stop=True,
    )
    H_T = const.tile([128, 256], f32)
    nc.scalar.copy(H_T[:], ps_h[:])

    # msg_nT[c, j] = (node @ Wn).T  (no bias: biases live in r0)
    ps_mn = ps_pool.tile([128, 256], f32, tag="ps")
    nc.tensor.matmul(
        ps_mn[:], W_sb["Wn"][:], nodeT[:],
        start=True, stop=True,
    )
    msg_nT = const.tile([128, 256], f32)
    nc.scalar.copy(msg_nT[:], ps_mn[:])

    # adjm = (adj - 1) * BIG  in {0, -BIG}, natural (i%128, (i//128, j)) layout
    adj_f = const.tile([128, NT * 256], f32)
    nc.vector.tensor_copy(adj_f[:], adj_nat[:])
    adjm = const.tile([128, NT * 256], f32r)
    nc.vector.tensor_scalar(adjm[:], adj_f[:], -1.0, BIG, Alu.add, Alu.mult)

    # cvec[j] = -BIG - max(sum_i adjm[i,j], -BIG)  -> -BIG if column fully
    # kept (k=256), else 0 (the "0 candidate" of the reference max)
    ps_s = ps_pool.tile([128, 256], f32, tag="ps")
    for t in range(NT):
        nc.tensor.matmul(
            ps_s[0:1, :],
            ones_col[:],
            adjm[:, t * 256 : (t + 1) * 256],
            start=(t == 0),
            stop=(t == NT - 1),
        )
    # cvec = -BIG if column fully kept (s == 0), else 0  (threshold form is
    # robust to f32r rounding of the BIG constants)
    cvec = const.tile([1, 256], f32r)
    nc.vector.tensor_scalar(cvec[:], ps_s[0:1, :], -1.0e29, -BIG, Alu.is_ge, Alu.mult)

    # running max accumulators (channels x j), round-robin x4 so consecutive
    # DVE ops never self-wait on the previous accumulation
    NACC = 8
    accs = []
    for q in range(NACC):
        a_ = const.tile([128, 256], f32, name=f"r{rep}_acc{q}", tag=f"acc{q}")
        nc.vector.memset(a_[:], -3.0e38)
        accs.append(a_)

    # ---- main loop over sender rows i -----------------------------------
    # Software-pipelined: group g's transposes+copies are emitted BEFORE
    # group g-1's matmuls+max ops, so the PE never stalls in-order on the
    # PE -> ACT(copy) -> PE round trip within one i.
    edge_r = edge.rearrange("i (t p) e -> p i t e", p=128)
    NG = N // GI

    def stage_a(g):
        """Load + transpose + evacuate group g; returns (128,512) eT pairs."""
        i0 = g * GI
        gsrc = g if edge_groups is None else (g % edge_groups)
        is0 = gsrc * GI
        et = epool.tile([128, GI * NT * 128], f32r, tag="et", name=f"r{rep}_et{g}")
        # alternate the two DMA issue rings so neither sequencer serializes
        dma_eng = nc.sync if g % 2 == 0 else nc.gpsimd
        dma_eng.dma_start(
            et[:].rearrange("p (a t e) -> p a t e", a=GI, t=NT),
            edge_r[:, is0 : is0 + GI, :, :].bitcast(f32r),
        )
        eTs = []
        for p in range(GI // 2):
            # one PSUM bank holds the transposes of two i values
            trp = trpool.tile([128, 512], f32r, tag="trp", name=f"r{rep}_trp{g}_{p}")
            for u in range(2):
                a = 2 * p + u
                for t in range(NT):
                    nc.tensor.transpose(
                        trp[:, (u * NT + t) * 128 : (u * NT + t + 1) * 128],
                        et[:, (a * NT + t) * 128 : (a * NT + t + 1) * 128],
                        ident_r[:],
                    )
            eT = etpool.tile([128, 512], f32r, tag="eT", name=f"r{rep}_eT{g}_{p}")
            nc.scalar.copy(eT[:], trp[:].bitcast(f32))
            eTs.append(eT)
        return eTs

    def stage_b(g, eTs, ar_chunk):
        """Matmuls + masked-max accumulation for group g (i pairs)."""
        i0 = g * GI
        for p in range(GI // 2):
            i = i0 + 2 * p
            op = opool.tile([128, 512], f32, tag="op", name=f"r{rep}_op{g}_{p}")
            nc.tensor.matmul(
                op[:], W_sb["We"][:], eTs[p][:],
                start=True, stop=False,
            )
            nc.tensor.matmul(
                op[:],
                ones_1c[:],
                ar_chunk[0:1, (i % CH) * 256 : (i % CH + 2) * 256],
                start=False,
                stop=True,
            )
            for u in range(2):
                iu = i + u
                a_ = accs[iu % NACC]
                nc.vector.scalar_tensor_tensor(
                    a_[:],
                    op[:, u * 256 : (u + 1) * 256],
                    H_T[:, iu : iu + 1],
                    a_[:],
                    Alu.add,
                    Alu.max,
                )

    def ar_stage(i0):
        """Stage adjm rows i0..i0+CH-1 at partition 0 (f32r, ACT HWDGE ring)."""
        ar = arpool.tile([1, CH * 256], f32r, tag="ar", name=f"r{rep}_ar{i0}")
        t_i = i0 // 128
        p0 = i0 % 128
        nc.scalar.dma_start(
            ar[:].rearrange("o (a j) -> o a j", a=CH),
            adjm[p0 : p0 + CH, t_i * 256 : (t_i + 1) * 256],
        )
        return ar

    GPC = CH // GI       # groups per adjm chunk
    prev = None          # (g, eTs)
    chunks = {0: ar_stage(0)}
    ar_chunk = chunks[0]
    for g in range(NG):
        # prefetch the next chunk halfway through the current one
        ck = (g * GI) // CH
        if g % GPC == GPC // 2 and (ck + 1) * CH < N:
            chunks[ck + 1] = ar_stage((ck + 1) * CH)
        eTs = stage_a(g)
        if prev is not None:
            stage_b(prev[0], prev[1], chunks[(prev[0] * GI) // CH])
        prev = (g, eTs)
    stage_b(prev[0], prev[1], chunks[(prev[0] * GI) // CH])

    # ---- finalize --------------------------------------------------------
    mrg = []
    for q in range(4):
        m_ = const.tile([128, 256], f32, name=f"r{rep}_mrg{q}", tag=f"mrg{q}")
        nc.vector.tensor_tensor(m_[:], accs[2 * q][:], accs[2 * q + 1][:], Alu.max)
        mrg.append(m_)
    a01 = const.tile([128, 256], f32)
    nc.vector.tensor_tensor(a01[:], mrg[0][:], mrg[1][:], Alu.max)
    a23 = const.tile([128, 256], f32)
    nc.vector.tensor_tensor(a23[:], mrg[2][:], mrg[3][:], Alu.max)
    acc = const.tile([128, 256], f32)
    nc.vector.tensor_tensor(acc[:], a01[:], a23[:], Alu.max)

    ps_cv = ps_pool.tile([128, 256], f32, tag="ps")
    nc.tensor.matmul(
        ps_cv[:], ones_1c[:], cvec[:],
        start=True, stop=True,
    )
    msgsT = const.tile([128, 256], f32)
    nc.vector.tensor_tensor(msgsT[:], acc[:], msg_nT[:], Alu.add)
    resT = const.tile([128, 256], f32r)
    nc.vector.tensor_tensor(resT[:], msgsT[:], ps_cv[:], Alu.max)

    # ret_T (o, n)
    ps_ret = ps_pool.tile([128, 256], f32, tag="ps")
    nc.tensor.matmul(
        ps_ret[:], W_sb["Wo1"][:], nodeT[:],
        start=True, stop=False,
    )
    nc.tensor.matmul(
        ps_ret[:], W_sb["Wo2"][:], hidT[:],
        start=False, stop=False,
    )
    nc.tensor.matmul(
        ps_ret[:], W_sb["Wo3"][:], resT[:],
        start=False, stop=False,
    )
    for k, bname in enumerate(["bo1", "bo2", "bo3"]):
        nc.tensor.matmul(
            ps_ret[:],
            B_sb[bname][:],
            ones_row[:],
            start=False,
            stop=(k == 2),
        )
    retT = const.tile([128, 256], f32)
    nc.scalar.copy(retT[:], ps_ret[:])

    ps_out = ps_pool.tile([128, 256], f32, tag="ps")
    for t in range(NT):
        nc.tensor.transpose(
            ps_out[:, t * 128 : (t + 1) * 128],
            retT[:, t * 128 : (t + 1) * 128],
            ident[:],
        )
    out_sb = const.tile([128, 256], f32)
    nc.scalar.copy(out_sb[:], ps_out[:])
    nc.sync.dma_start(
        out.rearrange("(t p) o -> p t o", p=128),
        out_sb[:].rearrange("p (t o) -> p t o", t=NT),
    )


def build_nc(repeat=1, edge_groups=None, loop_n=1):
    """Build the (single-core SPMD) Bass program; returns nc."""
    _ensure_path()
    import concourse.tile as tile
    from concourse import bacc, mybir

    f32 = mybir.dt.float32
    i32 = mybir.dt.int32

    nc = bacc.Bacc(
        "TRN2", target_bir_lowering=False, debug=False, num_devices=NCORES
    )
    n_edge_rows = N if edge_groups is None else edge_groups * GI
    aps = {
        "edge": nc.dram_tensor(
            "edge", [n_edge_rows, N, E], f32, kind="ExternalInput"
        ).ap(),
        "node": nc.dram_tensor("node", [N, D], f32, kind="ExternalInput").ap(),
        "hidden": nc.dram_tensor("hidden", [N, D], f32, kind="ExternalInput").ap(),
        "graph": nc.dram_tensor("graph", [G], f32, kind="ExternalInput").ap(),
        "adj": nc.dram_tensor("adj", [N, N], i32, kind="ExternalInput").ap(),
        "out": nc.dram_tensor("out", [N, OUT], f32, kind="ExternalOutput").ap(),
    }
    for w in _WNAMES:
        aps[w] = nc.dram_tensor(w, [128, 128], f32, kind="ExternalInput").ap()
    for b in _BNAMES:
        aps[b] = nc.dram_tensor(b, [128], f32, kind="ExternalInput").ap()

    with tile.TileContext(nc) as tc:
        if loop_n > 1:
            with tc.For_i(0, loop_n, 1):
                with ExitStack() as ctx:
                    _kernel_body(ctx, tc, aps, rep=0, edge_groups=edge_groups)
        else:
            for rep in range(repeat):
                with ExitStack() as ctx:
                    _kernel_body(ctx, tc, aps, rep=rep, edge_groups=edge_groups)
    nc.compile()
    return nc


def _get_nc():
    if "nc" not in _CACHE:
        _CACHE["nc"] = build_nc()
    return _CACHE["nc"]


def make_in_maps(**inputs):
    """Shard full inputs into per-core input maps (batch-parallel)."""
    in_maps = []
    for c in range(NCORES):
        m = {
            "edge": np.ascontiguousarray(inputs["edge_fts"][c], np.float32),
            "node": np.ascontiguousarray(inputs["node_fts"][c], np.float32),
            "hidden": np.ascontiguousarray(inputs["hidden"][c], np.float32),
            "graph": np.ascontiguousarray(inputs["graph_fts"][c], np.float32),
            "adj": np.ascontiguousarray(inputs["adj_mat"][c], np.int32),
        }
        for w in _WNAMES:
            m[w] = np.ascontiguousarray(inputs[w], np.float32)
        for b in _BNAMES:
            m[b] = np.ascontiguousarray(inputs[b], np.float32)
        in_maps.append(m)
    return in_maps


def kernel(**inputs) -> np.ndarray:
    """Full-input entry point: shards over 8 cores, returns (B, N, OUT)."""
    _ensure_path()
    from concourse import bass_utils

    nc = _get_nc()
    in_maps = make_in_maps(**inputs)
    res = bass_utils.run_bass_kernel_spmd(nc, in_maps, core_ids=list(range(NCORES)))
    outs = [res.results[c]["out"] for c in range(NCORES)]
    return np.stack(outs, axis=0).astype(np.float32)


def kernel_traced(tmpdir=None, **inputs):
    """Like kernel(), but requests an NTFF profile; returns (out, results)."""
    _ensure_path()
    from concourse import bass_utils

    nc = _get_nc()
    in_maps = make_in_maps(**inputs)
    res = bass_utils.run_bass_kernel_spmd(
        nc, in_maps, core_ids=list(range(NCORES)), trace=True, tmpdir=tmpdir
    )
    outs = [res.results[c]["out"] for c in range(NCORES)]
    return np.stack(outs, axis=0).astype(np.float32), res


if __name__ == "__main__":
    rng = np.random.default_rng(0)
    inputs = {
        "node_fts": rng.normal(size=(B, N, D)).astype(np.float32),
        "edge_fts": rng.normal(size=(B, N, N, E)).astype(np.float32),
        "graph_fts": rng.normal(size=(B, G)).astype(np.float32),
        "adj_mat": rng.integers(0, 2, size=(B, N, N)).astype(np.int32),
        "hidden": rng.normal(size=(B, N, D)).astype(np.float32),
    }
    s = 0.02
    for w in _WNAMES:
        inputs[w] = (s * rng.normal(size=(128, 128))).astype(np.float32)
    for b in _BNAMES:
        inputs[b] = np.zeros(128, np.float32)
    out = kernel(**inputs)
    print(out.shape, out.dtype)


# revision 21
# speedup vs baseline: 2.2616x; 1.2796x over previous
"""Trainium2 Bass kernel for the GNN message-passing module.

Reference computation (per batch b):
    msg_n = node @ Wn + bn                      (N, MID)
    msg_h = hidden @ Wh + bh                    (N, MID)
    msg_e = edge @ We + be                      (N, N, MID)
    msg_g = graph @ Wg + bg                     (MID,)
    msgs[i,j,:] = msg_n[j] + msg_h[i] + msg_e[i,j] + msg_g
    out_msgs[j,:] = max_i(msgs[i,j,:] * adj[i,j])
    ret = node @ Wo1 + bo1 + hidden @ Wo2 + bo2 + out_msgs @ Wo3 + bo3

Kernel strategy (data-parallel, one batch per core across 8 cores):
  - Orientation: channels on SBUF partitions, j (receiver) on the free dim.
  - The multiplicative {0,1} adjacency mask is converted to an additive mask
    adjm = (adj-1)*1e30 in {0, -1e30}, folded into the PE accumulation as a
    rank-1 matmul (ones_c (x) adjm_row_i).  A per-j correction vector cvec
    restores the exact max semantics (masked entries contribute 0 to the max,
    all-kept columns must not see the 0 candidate).
  - msg_n is constant in i, so it is pulled out of the max and added once.
  - h_i = msg_h[i] + msg_g + (bn+bh+be+bg) enters through the fused DVE op
    acc = max(acc, psum_i + h_col_i) (scalar_tensor_tensor, one op per i).
  - fp32 data is fed to the PE as float32r (replicated fp32), which streams at
    1 cycle/row for free dims >= 256 while keeping full fp32 precision.
"""

from contextlib import ExitStack

import numpy as np

B, N, D, E, G, MID, OUT = 8, 256, 128, 128, 128, 128, 128
NCORES = 8
BIG = 1.0e30
GI = 8  # edge rows (i values) per DMA group
CH = 32  # adjm rows per staging chunk
NT = N // 128  # number of 128-row tiles along N

_WNAMES = ["Wn", "Wh", "We", "Wg", "Wo1", "Wo2", "Wo3"]
_BNAMES = ["bn", "bh", "be", "bg", "bo1", "bo2", "bo3"]

_CACHE = {}


def _ensure_path():
    try:
        import concourse.bass  # noqa: F401
    except ImportError:
        import sys

        for p in ("/opt/trn_rl_repo", "/root/.axon_site/_ro/trn_rl_repo"):
            if p not in sys.path:
                sys.path.insert(0, p)
        import concourse.bass  # noqa: F401


def _kernel_body(ctx, tc, aps, rep=0, edge_groups=None):
    import concourse.bass as bass  # noqa: F401
    from concourse import masks, mybir

    nc = tc.nc
    f32 = mybir.dt.float32
    f32r = mybir.dt.float32r
    Alu = mybir.AluOpType

    edge = aps["edge"]
    node = aps["node"]
    hidden = aps["hidden"]
    graph = aps["graph"]
    adj = aps["adj"]
    out = aps["out"]

    const = ctx.enter_context(tc.tile_pool(name="const", bufs=1))
    ps_pool = ctx.enter_context(tc.tile_pool(name="ps", bufs=2, space="PSUM"))
    opool = ctx.enter_context(tc.tile_pool(name="op", bufs=6, space="PSUM"))
    epool = ctx.enter_context(tc.tile_pool(name="edgein", bufs=3))
    tpool = ctx.enter_context(tc.tile_pool(name="tsb", bufs=6))
    arpool = ctx.enter_context(tc.tile_pool(name="adjrow", bufs=2))

    # ---- constants -------------------------------------------------------
    ident = const.tile([128, 128], f32)
    masks.make_identity(nc, ident[:])

    ones_f = const.tile([1, 256], f32)
    nc.vector.memset(ones_f[:], 1.0)
    ones_row = const.tile([1, 256], f32r)
    nc.vector.tensor_copy(ones_row[:], ones_f[:])
    ones_1c = const.tile([1, 128], f32r)
    nc.vector.tensor_copy(ones_1c[:], ones_f[:, 0:128])
    ones_11 = const.tile([1, 1], f32r)
    nc.vector.tensor_copy(ones_11[:], ones_f[:, 0:1])
    ones_colf = const.tile([128, 1], f32)
    nc.vector.memset(ones_colf[:], 1.0)
    ones_col = const.tile([128, 1], f32r)
    nc.vector.tensor_copy(ones_col[:], ones_colf[:])

    W_sb = {}
    for w in _WNAMES:
        Wf = const.tile([128, 128], f32, name=f"r{rep}_Wf_{w}", tag=f"Wf_{w}")
        nc.sync.dma_start(Wf[:], aps[w])
        W_sb[w] = const.tile([128, 128], f32r, name=f"r{rep}_W_{w}", tag=f"W_{w}")
        nc.vector.tensor_copy(W_sb[w][:], Wf[:])
    B_sb = {}
    for b in _BNAMES:
        Bf = const.tile([1, 128], f32, name=f"r{rep}_Bf_{b}", tag=f"Bf_{b}")
        nc.sync.dma_start(Bf[:], aps[b].rearrange("(o k) -> o k", o=1))
        B_sb[b] = const.tile([1, 128], f32r, name=f"r{rep}_B_{b}", tag=f"B_{b}")
        nc.vector.tensor_copy(B_sb[b][:], Bf[:])

    graph_colf = const.tile([128, 1], f32)
    nc.sync.dma_start(graph_colf[:], graph.rearrange("(p o) -> p o", o=1))
    graph_col = const.tile([128, 1], f32r)
    nc.vector.tensor_copy(graph_col[:], graph_colf[:])

    node_nat = const.tile([128, NT * 128], f32)
    nc.sync.dma_start(
        node_nat[:].rearrange("p (t d) -> p t d", t=NT),
        node.rearrange("(t p) d -> p t d", p=128),
    )
    hid_nat = const.tile([128, NT * 128], f32)
    nc.sync.dma_start(
        hid_nat[:].rearrange("p (t d) -> p t d", t=NT),
        hidden.rearrange("(t p) d -> p t d", p=128),
    )
    adj_nat = const.tile([128, NT * 256], mybir.dt.int32)
    nc.sync.dma_start(
        adj_nat[:].rearrange("p (t j) -> p t j", t=NT),
        adj.rearrange("(t p) j -> p t j", p=128),
    )

    # ---- per-batch precompute -------------------------------------------
    # nodeT / hidT: (d, n) layouts via PE transpose
    nodeT = const.tile([128, 256], f32r)
    hidT = const.tile([128, 256], f32r)
    for nat, T in ((node_nat, nodeT), (hid_nat, hidT)):
        ps = ps_pool.tile([128, 256], f32, tag="ps")
        for t in range(NT):
            nc.tensor.transpose(
                ps[:, t * 128 : (t + 1) * 128],
                nat[:, t * 128 : (t + 1) * 128],
                ident[:],
            )
        nc.scalar.copy(T[:], ps[:])

    # r0 = graph @ Wg + (bn + bh + be + bg), a (1, MID) row
    ps_r0 = ps_pool.tile([128, 256], f32, tag="ps")
    nc.tensor.matmul(
        ps_r0[0:1, 0:128],
        graph_col[:],
        W_sb["Wg"][:],
        start=True,
        stop=False,
    )
    for k, bname in enumerate(["bn", "bh", "be", "bg"]):
        nc.tensor.matmul(
            ps_r0[0:1, 0:128],
            ones_11[:],
            B_sb[bname][:],
            start=False,
            stop=(k == 3),
        )
    r0 = const.tile([1, 128], f32r)
    nc.scalar.copy(r0[:], ps_r0[0:1, 0:128])

    # H_T[c, i] = (hidden @ Wh).T + r0[c]  (h_i rows, channel-major)
    ps_h = ps_pool.tile([128, 256], f32, tag="ps")
    nc.tensor.matmul(
        ps_h[:], W_sb["Wh"][:], hidT[:],
        start=True, stop=False,
    )
    nc.tensor.matmul(
        ps_h[:], r0[:], ones_row[:],
        start=False, stop=True,
    )
    H_T = const.tile([128, 256], f32)
    nc.scalar.copy(H_T[:], ps_h[:])

    # msg_nT[c, j] = (node @ Wn).T  (no bias: biases live in r0)
    ps_mn = ps_pool.tile([128, 256], f32, tag="ps")
    nc.tensor.matmul(
        ps_mn[:], W_sb["Wn"][:], nodeT[:],
        start=True, stop=True,
    )
    msg_nT = const.tile([128, 256], f32)
    nc.scalar.copy(msg_nT[:], ps_mn[:])

    # adjm = (adj - 1) * BIG  in {0, -BIG}, natural (i%128, (i//128, j)) layout
    adj_f = const.tile([128, NT * 256], f32)
    nc.vector.tensor_copy(adj_f[:], adj_nat[:])
    adjm = const.tile([128, NT * 256], f32r)
    nc.vector.tensor_scalar(adjm[:], adj_f[:], -1.0, BIG, Alu.add, Alu.mult)

    # cvec[j] = -BIG - max(sum_i adjm[i,j], -BIG)  -> -BIG if column fully
    # kept (k=256), else 0 (the "0 candidate" of the reference max)
    ps_s = ps_pool.tile([128, 256], f32, tag="ps")
    for t in range(NT):
        nc.tensor.matmul(
            ps_s[0:1, :],
            ones_col[:],
            adjm[:, t * 256 : (t + 1) * 256],
            start=(t == 0),
            stop=(t == NT - 1),
        )
    # cvec = -BIG if column fully kept (s == 0), else 0  (threshold form is
    # robust to f32r rounding of the BIG constants)
    cvec = const.tile([1, 256], f32r)
    nc.vector.tensor_scalar(cvec[:], ps_s[0:1, :], -1.0e29, -BIG, Alu.is_ge, Alu.mult)

    # running max accumulators (channels x (pair, j)), round-robin so
    # consecutive DVE ops never self-wait on the previous accumulation
    NACC = 4
    accs = []
    for q in range(NACC):
        a_ = const.tile([128, 512], f32, name=f"r{rep}_acc{q}", tag=f"acc{q}")
        nc.vector.memset(a_[:], -3.0e38)
        accs.append(a_)

    # ---- main loop over sender rows i -----------------------------------
    # Edge arrives pre-transposed from the host as (i, e, j): tiles load
    # directly in matmul orientation (e on partitions).  Per i-pair:
    # PE: 2 matmuls into one PSUM bank; ACT: per-half bias-add (h_i)
    # evacuation to SBUF; DVE: one wide (128,512) running max.
    edge_r = edge.rearrange("i e j -> e i j")
    NG = N // GI

    def stage_a(g):
        """Load group g; returns the (e, (a, j)) tile."""
        i0 = g * GI
        gsrc = g if edge_groups is None else (g % edge_groups)
        is0 = gsrc * GI
        et = epool.tile([128, GI * 256], f32r, tag="et", name=f"r{rep}_et{g}")
        # alternate the two DMA issue rings so neither sequencer serializes
        dma_eng = nc.sync if g % 2 == 0 else nc.gpsimd
        dma_eng.dma_start(
            et[:].rearrange("p (a j) -> p a j", a=GI),
            edge_r[:, is0 : is0 + GI, :].bitcast(f32r),
        )
        return et

    def stage_b(g, et, ar_chunk):
        """Matmuls + bias-evac + masked-max accumulation for group g."""
        i0 = g * GI
        for p in range(GI // 2):
            i = i0 + 2 * p
            op = opool.tile([128, 512], f32, tag="op", name=f"r{rep}_op{g}_{p}")
            nc.tensor.matmul(
                op[:], W_sb["We"][:], et[:, (2 * p) * 256 : (2 * p + 2) * 256],
                start=True, stop=False,
            )
            nc.tensor.matmul(
                op[:],
                ones_1c[:],
                ar_chunk[0:1, (i % CH) * 256 : (i % CH + 2) * 256],
                start=False,
                stop=True,
            )
            t_ = tpool.tile([128, 512], f32, tag="t", name=f"r{rep}_t{g}_{p}")
            for u in range(2):
                iu = i + u
                nc.scalar.activation(
                    t_[:, u * 256 : (u + 1) * 256],
                    op[:, u * 256 : (u + 1) * 256],
                    mybir.ActivationFunctionType.Identity,
                    bias=H_T[:, iu : iu + 1],
                )
            a_ = accs[(g * (GI // 2) + p) % NACC]
            nc.vector.tensor_tensor(a_[:], t_[:], a_[:], Alu.max)

    def ar_stage(i0):
        """Stage adjm rows i0..i0+CH-1 at partition 0 (f32r, ACT HWDGE ring)."""
        ar = arpool.tile([1, CH * 256], f32r, tag="ar", name=f"r{rep}_ar{i0}")
        t_i = i0 // 128
        p0 = i0 % 128
        nc.scalar.dma_start(
            ar[:].rearrange("o (a j) -> o a j", a=CH),
            adjm[p0 : p0 + CH, t_i * 256 : (t_i + 1) * 256],
        )
        return ar

    GPC = CH // GI       # groups per adjm chunk
    prev = None          # (g, eTs)
    chunks = {0: ar_stage(0)}
    ar_chunk = chunks[0]
    for g in range(NG):
        # prefetch the next chunk halfway through the current one
        ck = (g * GI) // CH
        if g % GPC == GPC // 2 and (ck + 1) * CH < N:
            chunks[ck + 1] = ar_stage((ck + 1) * CH)
        et = stage_a(g)
        if prev is not None:
            stage_b(prev[0], prev[1], chunks[(prev[0] * GI) // CH])
        prev = (g, et)
    stage_b(prev[0], prev[1], chunks[(prev[0] * GI) // CH])

    # ---- finalize --------------------------------------------------------
    a01 = const.tile([128, 512], f32)
    nc.vector.tensor_tensor(a01[:], accs[0][:], accs[1][:], Alu.max)
    a23 = const.tile([128, 512], f32)
    nc.vector.tensor_tensor(a23[:], accs[2][:], accs[3][:], Alu.max)
    aw = const.tile([128, 512], f32)
    nc.vector.tensor_tensor(aw[:], a01[:], a23[:], Alu.max)
    acc = const.tile([128, 256], f32)
    nc.vector.tensor_tensor(acc[:], aw[:, 0:256], aw[:, 256:512], Alu.max)

    ps_cv = ps_pool.tile([128, 256], f32, tag="ps")
    nc.tensor.matmul(
        ps_cv[:], ones_1c[:], cvec[:],
        start=True, stop=True,
    )
    msgsT = const.tile([128, 256], f32)
    nc.vector.tensor_tensor(msgsT[:], acc[:], msg_nT[:], Alu.add)
    resT = const.tile([128, 256], f32r)
    nc.vector.tensor_tensor(resT[:], msgsT[:], ps_cv[:], Alu.max)

    # ret_T (o, n)
    ps_ret = ps_pool.tile([128, 256], f32, tag="ps")
    nc.tensor.matmul(
        ps_ret[:], W_sb["Wo1"][:], nodeT[:],
        start=True, stop=False,
    )
    nc.tensor.matmul(
        ps_ret[:], W_sb["Wo2"][:], hidT[:],
        start=False, stop=False,
    )
    nc.tensor.matmul(
        ps_ret[:], W_sb["Wo3"][:], resT[:],
        start=False, stop=False,
    )
    for k, bname in enumerate(["bo1", "bo2", "bo3"]):
        nc.tensor.matmul(
            ps_ret[:],
            B_sb[bname][:],
            ones_row[:],
            start=False,
            stop=(k == 2),
        )
    retT = const.tile([128, 256], f32)
    nc.scalar.copy(retT[:], ps_ret[:])

    ps_out = ps_pool.tile([128, 256], f32, tag="ps")
    for t in range(NT):
        nc.tensor.transpose(
            ps_out[:, t * 128 : (t + 1) * 128],
            retT[:, t * 128 : (t + 1) * 128],
            ident[:],
        )
    out_sb = const.tile([128, 256], f32)
    nc.scalar.copy(out_sb[:], ps_out[:])
    nc.sync.dma_start(
        out.rearrange("(t p) o -> p t o", p=128),
        out_sb[:].rearrange("p (t o) -> p t o", t=NT),
    )


def build_nc(repeat=1, edge_groups=None, loop_n=1):
    """Build the (single-core SPMD) Bass program; returns nc."""
    _ensure_path()
    import concourse.tile as tile
    from concourse import bacc, mybir

    f32 = mybir.dt.float32
    i32 = mybir.dt.int32

    nc = bacc.Bacc(
        "TRN2", target_bir_lowering=False, debug=False, num_devices=NCORES
    )
    n_edge_rows = N if edge_groups is None else edge_groups * GI
    aps = {
        "edge": nc.dram_tensor(
            "edge", [n_edge_rows, E, N], f32, kind="ExternalInput"
        ).ap(),
        "node": nc.dram_tensor("node", [N, D], f32, kind="ExternalInput").ap(),
        "hidden": nc.dram_tensor("hidden", [N, D], f32, kind="ExternalInput").ap(),
        "graph": nc.dram_tensor("graph", [G], f32, kind="ExternalInput").ap(),
        "adj": nc.dram_tensor("adj", [N, N], i32, kind="ExternalInput").ap(),
        "out": nc.dram_tensor("out", [N, OUT], f32, kind="ExternalOutput").ap(),
    }
    for w in _WNAMES:
        aps[w] = nc.dram_tensor(w, [128, 128], f32, kind="ExternalInput").ap()
    for b in _BNAMES:
        aps[b] = nc.dram_tensor(b, [128], f32, kind="ExternalInput").ap()

    with tile.TileContext(nc) as tc:
        if loop_n > 1:
            with tc.For_i(0, loop_n, 1):
                with ExitStack() as ctx:
                    _kernel_body(ctx, tc, aps, rep=0, edge_groups=edge_groups)
        else:
            for rep in range(repeat):
                with ExitStack() as ctx:
                    _kernel_body(ctx, tc, aps, rep=rep, edge_groups=edge_groups)
    nc.compile()
    return nc


def _get_nc():
    if "nc" not in _CACHE:
        _CACHE["nc"] = build_nc()
    return _CACHE["nc"]


def make_in_maps(**inputs):
    """Shard full inputs into per-core input maps (batch-parallel)."""
    in_maps = []
    for c in range(NCORES):
        m = {
            "edge": np.ascontiguousarray(
                np.asarray(inputs["edge_fts"][c], np.float32).transpose(0, 2, 1)
            ),
            "node": np.ascontiguousarray(inputs["node_fts"][c], np.float32),
            "hidden": np.ascontiguousarray(inputs["hidden"][c], np.float32),
            "graph": np.ascontiguousarray(inputs["graph_fts"][c], np.float32),
            "adj": np.ascontiguousarray(inputs["adj_mat"][c], np.int32),
        }
        for w in _WNAMES:
            m[w] = np.ascontiguousarray(inputs[w], np.float32)
        for b in _BNAMES:
            m[b] = np.ascontiguousarray(inputs[b], np.float32)
        in_maps.append(m)
    return in_maps


def kernel(**inputs) -> np.ndarray:
    """Full-input entry point: shards over 8 cores, returns (B, N, OUT)."""
    _ensure_path()
    from concourse import bass_utils

    nc = _get_nc()
    in_maps = make_in_maps(**inputs)
    res = bass_utils.run_bass_kernel_spmd(nc, in_maps, core_ids=list(range(NCORES)))
    outs = [res.results[c]["out"] for c in range(NCORES)]
    return np.stack(outs, axis=0).astype(np.float32)


def kernel_traced(tmpdir=None, **inputs):
    """Like kernel(), but requests an NTFF profile; returns (out, results)."""
    _ensure_path()
    from concourse import bass_utils

    nc = _get_nc()
    in_maps = make_in_maps(**inputs)
    res = bass_utils.run_bass_kernel_spmd(
        nc, in_maps, core_ids=list(range(NCORES)), trace=True, tmpdir=tmpdir
    )
    outs = [res.results[c]["out"] for c in range(NCORES)]
    return np.stack(outs, axis=0).astype(np.float32), res


if __name__ == "__main__":
    rng = np.random.default_rng(0)
    inputs = {
        "node_fts": rng.normal(size=(B, N, D)).astype(np.float32),
        "edge_fts": rng.normal(size=(B, N, N, E)).astype(np.float32),
        "graph_fts": rng.normal(size=(B, G)).astype(np.float32),
        "adj_mat": rng.integers(0, 2, size=(B, N, N)).astype(np.int32),
        "hidden": rng.normal(size=(B, N, D)).astype(np.float32),
    }
    s = 0.02
    for w in _WNAMES:
        inputs[w] = (s * rng.normal(size=(128, 128))).astype(np.float32)
    for b in _BNAMES:
        inputs[b] = np.zeros(128, np.float32)
    out = kernel(**inputs)
    print(out.shape, out.dtype)


# revision 23
# speedup vs baseline: 2.6394x; 1.1671x over previous
"""Trainium2 Bass kernel for the GNN message-passing module.

Reference computation (per batch b):
    msg_n = node @ Wn + bn                      (N, MID)
    msg_h = hidden @ Wh + bh                    (N, MID)
    msg_e = edge @ We + be                      (N, N, MID)
    msg_g = graph @ Wg + bg                     (MID,)
    msgs[i,j,:] = msg_n[j] + msg_h[i] + msg_e[i,j] + msg_g
    out_msgs[j,:] = max_i(msgs[i,j,:] * adj[i,j])
    ret = node @ Wo1 + bo1 + hidden @ Wo2 + bo2 + out_msgs @ Wo3 + bo3

Kernel strategy (data-parallel, one batch per core across 8 cores):
  - Orientation: channels on SBUF partitions, j (receiver) on the free dim.
  - The multiplicative {0,1} adjacency mask is converted to an additive mask
    adjm = (adj-1)*1e30 in {0, -1e30}, folded into the PE accumulation as a
    rank-1 matmul (ones_c (x) adjm_row_i).  A per-j correction vector cvec
    restores the exact max semantics (masked entries contribute 0 to the max,
    all-kept columns must not see the 0 candidate).
  - msg_n is constant in i, so it is pulled out of the max and added once.
  - h_i = msg_h[i] + msg_g + (bn+bh+be+bg) enters through the fused DVE op
    acc = max(acc, psum_i + h_col_i) (scalar_tensor_tensor, one op per i).
  - fp32 data is fed to the PE as float32r (replicated fp32), which streams at
    1 cycle/row for free dims >= 256 while keeping full fp32 precision.
"""

from contextlib import ExitStack

import numpy as np

B, N, D, E, G, MID, OUT = 8, 256, 128, 128, 128, 128, 128
NCORES = 8
BIG = 1.0e30
GI = 8  # edge rows (i values) per DMA group
CH = 8   # staging chunk == one edge group; pairs are (i, i+4)
NT = N // 128  # number of 128-row tiles along N

_WNAMES = ["Wn", "Wh", "We", "Wg", "Wo1", "Wo2", "Wo3"]
_BNAMES = ["bn", "bh", "be", "bg", "bo1", "bo2", "bo3"]

_CACHE = {}


def _ensure_path():
    try:
        import concourse.bass  # noqa: F401
    except ImportError:
        import sys

        for p in ("/opt/trn_rl_repo", "/root/.axon_site/_ro/trn_rl_repo"):
            if p not in sys.path:
                sys.path.insert(0, p)
        import concourse.bass  # noqa: F401


def _kernel_body(ctx, tc, aps, rep=0, edge_groups=None):
    import concourse.bass as bass  # noqa: F401
    from concourse import masks, mybir

    nc = tc.nc
    f32 = mybir.dt.float32
    f32r = mybir.dt.float32r
    Alu = mybir.AluOpType

    edge = aps["edge"]
    node = aps["node"]
    hidden = aps["hidden"]
    graph = aps["graph"]
    adj = aps["adj"]
    out = aps["out"]

    const = ctx.enter_context(tc.tile_pool(name="const", bufs=1))
    ps_pool = ctx.enter_context(tc.tile_pool(name="ps", bufs=2, space="PSUM"))
    opool = ctx.enter_context(tc.tile_pool(name="op", bufs=6, space="PSUM"))
    epool = ctx.enter_context(tc.tile_pool(name="edgein", bufs=3))

    # ---- constants -------------------------------------------------------
    ident = const.tile([128, 128], f32)
    masks.make_identity(nc, ident[:])

    ones_f = const.tile([1, 256], f32)
    nc.vector.memset(ones_f[:], 1.0)
    ones_row = const.tile([1, 256], f32r)
    nc.vector.tensor_copy(ones_row[:], ones_f[:])
    ones_1c = const.tile([1, 128], f32r)
    nc.vector.tensor_copy(ones_1c[:], ones_f[:, 0:128])
    ones_11 = const.tile([1, 1], f32r)
    nc.vector.tensor_copy(ones_11[:], ones_f[:, 0:1])
    ones_colf = const.tile([128, 1], f32)
    nc.vector.memset(ones_colf[:], 1.0)
    ones_col = const.tile([128, 1], f32r)
    nc.vector.tensor_copy(ones_col[:], ones_colf[:])

    W_sb = {}
    for w in _WNAMES:
        Wf = const.tile([128, 128], f32, name=f"r{rep}_Wf_{w}", tag=f"Wf_{w}")
        nc.sync.dma_start(Wf[:], aps[w])
        W_sb[w] = const.tile([128, 128], f32r, name=f"r{rep}_W_{w}", tag=f"W_{w}")
        nc.vector.tensor_copy(W_sb[w][:], Wf[:])
    B_sb = {}
    for b in _BNAMES:
        Bf = const.tile([1, 128], f32, name=f"r{rep}_Bf_{b}", tag=f"Bf_{b}")
        nc.sync.dma_start(Bf[:], aps[b].rearrange("(o k) -> o k", o=1))
        B_sb[b] = const.tile([1, 128], f32r, name=f"r{rep}_B_{b}", tag=f"B_{b}")
        nc.vector.tensor_copy(B_sb[b][:], Bf[:])

    graph_colf = const.tile([128, 1], f32)
    nc.sync.dma_start(graph_colf[:], graph.rearrange("(p o) -> p o", o=1))
    graph_col = const.tile([128, 1], f32r)
    nc.vector.tensor_copy(graph_col[:], graph_colf[:])

    node_nat = const.tile([128, NT * 128], f32)
    nc.sync.dma_start(
        node_nat[:].rearrange("p (t d) -> p t d", t=NT),
        node.rearrange("(t p) d -> p t d", p=128),
    )
    hid_nat = const.tile([128, NT * 128], f32)
    nc.sync.dma_start(
        hid_nat[:].rearrange("p (t d) -> p t d", t=NT),
        hidden.rearrange("(t p) d -> p t d", p=128),
    )
    adj_nat = const.tile([128, NT * 256], mybir.dt.int32)
    nc.sync.dma_start(
        adj_nat[:].rearrange("p (t j) -> p t j", t=NT),
        adj.rearrange("(t p) j -> p t j", p=128),
    )

    # ---- per-batch precompute -------------------------------------------
    # nodeT / hidT: (d, n) layouts via PE transpose
    nodeT = const.tile([128, 256], f32r)
    hidT = const.tile([128, 256], f32r)
    for nat, T in ((node_nat, nodeT), (hid_nat, hidT)):
        ps = ps_pool.tile([128, 256], f32, tag="ps")
        for t in range(NT):
            nc.tensor.transpose(
                ps[:, t * 128 : (t + 1) * 128],
                nat[:, t * 128 : (t + 1) * 128],
                ident[:],
            )
        nc.scalar.copy(T[:], ps[:])

    # r0 = graph @ Wg + (bn + bh + be + bg), a (1, MID) row
    ps_r0 = ps_pool.tile([128, 256], f32, tag="ps")
    nc.tensor.matmul(
        ps_r0[0:1, 0:128],
        graph_col[:],
        W_sb["Wg"][:],
        start=True,
        stop=False,
    )
    for k, bname in enumerate(["bn", "bh", "be", "bg"]):
        nc.tensor.matmul(
            ps_r0[0:1, 0:128],
            ones_11[:],
            B_sb[bname][:],
            start=False,
            stop=(k == 3),
        )
    r0 = const.tile([1, 128], f32r)
    nc.scalar.copy(r0[:], ps_r0[0:1, 0:128])

    # H_nat[i, c] = hidden @ Wh + r0  (h_i rows, natural orientation, f32r)
    ps_h = ps_pool.tile([128, 256], f32, tag="ps")
    for t in range(NT):
        nc.tensor.matmul(
            ps_h[:, t * 128 : (t + 1) * 128],
            hidT[:, t * 128 : (t + 1) * 128],
            W_sb["Wh"][:],
            start=True,
            stop=False,
        )
        nc.tensor.matmul(
            ps_h[:, t * 128 : (t + 1) * 128],
            ones_1c[:],
            r0[:],
            start=False,
            stop=True,
        )
    H_natr = const.tile([128, 256], f32r)
    nc.scalar.copy(H_natr[:], ps_h[:])

    # Persistent double-buffered staging tiles for the fused K=3 matmul:
    #   Hab rows: [h_lo; h_hi; ones]   AR3 rows: [sel0; sel1; adjm pairs]
    # pair q of group g = (i0+q, i0+q+4)
    PPC = CH // 2
    habA = const.tile([3, PPC * 128], f32r)
    habB = const.tile([3, PPC * 128], f32r)
    arA = const.tile([3, PPC * 512], f32r)
    arB = const.tile([3, PPC * 512], f32r)
    sel0f = const.tile([1, PPC * 512], f32)
    nc.vector.memset(sel0f[:], 0.0)
    nc.vector.memset(
        sel0f[:].rearrange("o (a u j) -> o a u j", a=PPC, u=2)[:, :, 0:1, :], 1.0
    )
    sel1f = const.tile([1, PPC * 512], f32)
    nc.vector.memset(sel1f[:], 0.0)
    nc.vector.memset(
        sel1f[:].rearrange("o (a u j) -> o a u j", a=PPC, u=2)[:, :, 1:2, :], 1.0
    )
    onesw_f = const.tile([1, PPC * 128], f32)
    nc.vector.memset(onesw_f[:], 1.0)
    for dst in (arA, arB):
        nc.gpsimd.dma_start(dst[0:1, :], sel0f[:])
        nc.gpsimd.dma_start(dst[1:2, :], sel1f[:])
    for dst in (habA, habB):
        nc.gpsimd.dma_start(dst[2:3, :], onesw_f[:])

    # msg_nT[c, j] = (node @ Wn).T  (no bias: biases live in r0)
    ps_mn = ps_pool.tile([128, 256], f32, tag="ps")
    nc.tensor.matmul(
        ps_mn[:], W_sb["Wn"][:], nodeT[:],
        start=True, stop=True,
    )
    msg_nT = const.tile([128, 256], f32)
    nc.scalar.copy(msg_nT[:], ps_mn[:])

    # adjm = (adj - 1) * BIG  in {0, -BIG}, natural (i%128, (i//128, j)) layout
    adj_f = const.tile([128, NT * 256], f32)
    nc.vector.tensor_copy(adj_f[:], adj_nat[:])
    adjm = const.tile([128, NT * 256], f32r)
    nc.vector.tensor_scalar(adjm[:], adj_f[:], -1.0, BIG, Alu.add, Alu.mult)

    # cvec[j] = -BIG - max(sum_i adjm[i,j], -BIG)  -> -BIG if column fully
    # kept (k=256), else 0 (the "0 candidate" of the reference max)
    ps_s = ps_pool.tile([128, 256], f32, tag="ps")
    for t in range(NT):
        nc.tensor.matmul(
            ps_s[0:1, :],
            ones_col[:],
            adjm[:, t * 256 : (t + 1) * 256],
            start=(t == 0),
            stop=(t == NT - 1),
        )
    # cvec = -BIG if column fully kept (s == 0), else 0  (threshold form is
    # robust to f32r rounding of the BIG constants)
    cvec = const.tile([1, 256], f32r)
    nc.vector.tensor_scalar(cvec[:], ps_s[0:1, :], -1.0e29, -BIG, Alu.is_ge, Alu.mult)

    # running max accumulators (channels x (pair, j)), round-robin so
    # consecutive DVE ops never self-wait on the previous accumulation
    NACC = 4
    accs = []
    for q in range(NACC):
        a_ = const.tile([128, 512], f32, name=f"r{rep}_acc{q}", tag=f"acc{q}")
        nc.vector.memset(a_[:], -3.0e38)
        accs.append(a_)

    # ---- main loop over sender rows i -----------------------------------
    # Edge arrives pre-transposed from the host as (i, e, j): tiles load
    # directly in matmul orientation (e on partitions).  Per i-pair:
    # PE: 2 matmuls into one PSUM bank; ACT: per-half bias-add (h_i)
    # evacuation to SBUF; DVE: one wide (128,512) running max.
    edge_r = edge.rearrange("i e j -> e i j")
    NG = N // GI

    def stage_a(g):
        """Load group g; returns the (e, (a, j)) tile."""
        i0 = g * GI
        gsrc = g if edge_groups is None else (g % edge_groups)
        is0 = gsrc * GI
        et = epool.tile([128, GI * 256], f32r, tag="et", name=f"r{rep}_et{g}")
        # alternate the two DMA issue rings so neither sequencer serializes
        dma_eng = nc.sync if g % 2 == 0 else nc.gpsimd
        dma_eng.dma_start(
            et[:].rearrange("p (a j) -> p a j", a=GI),
            edge_r[:, is0 : is0 + GI, :].bitcast(f32r),
        )
        return et

    def stage_b(g, et, chunk):
        """msg_e matmul + fused (h, adjm) rank-3 matmul + wide running max."""
        AR3, Hab = chunk
        et_r = et[:].rearrange("p (u q j) -> p u q j", u=2, q=PPC)
        for q in range(PPC):
            op = opool.tile([128, 512], f32, tag="op", name=f"r{rep}_op{g}_{q}")
            nc.tensor.matmul(
                op[:].rearrange("p (u j) -> p u j", u=2),
                W_sb["We"][:],
                et_r[:, :, q, :],
                start=True, stop=False,
            )
            nc.tensor.matmul(
                op[:],
                Hab[0:3, q * 128 : (q + 1) * 128],
                AR3[0:3, q * 512 : (q + 1) * 512],
                start=False,
                stop=True,
            )
            a_ = accs[(g * PPC + q) % NACC]
            nc.vector.tensor_tensor(a_[:], op[:], a_[:], Alu.max)

    def ar_stage(i0):
        """Stage adjm rows + h pairs for group starting at i0 (ACT ring)."""
        k = i0 // CH
        AR3, Hab = (arA, habA) if k % 2 == 0 else (arB, habB)
        t_i = i0 // 128
        p0 = i0 % 128
        ar_v = AR3[2:3, :].rearrange("o (q u j) -> o q u j", q=PPC, u=2)
        for u in range(2):
            nc.scalar.dma_start(
                ar_v[:, :, u, :],
                adjm[p0 + 4 * u : p0 + 4 * u + 4, t_i * 256 : (t_i + 1) * 256],
            )
            nc.scalar.dma_start(
                Hab[u : u + 1, :].rearrange("o (q c) -> o q c", q=PPC),
                H_natr[p0 + 4 * u : p0 + 4 * u + 4, t_i * 128 : (t_i + 1) * 128],
            )
        return (AR3, Hab)

    prev = None          # (g, et, chunk)
    for g in range(NG):
        ck = ar_stage(g * GI)
        et = stage_a(g)
        if prev is not None:
            stage_b(prev[0], prev[1], prev[2])
        prev = (g, et, ck)
    stage_b(prev[0], prev[1], prev[2])

    # ---- finalize --------------------------------------------------------
    a01 = const.tile([128, 512], f32)
    nc.vector.tensor_tensor(a01[:], accs[0][:], accs[1][:], Alu.max)
    a23 = const.tile([128, 512], f32)
    nc.vector.tensor_tensor(a23[:], accs[2][:], accs[3][:], Alu.max)
    aw = const.tile([128, 512], f32)
    nc.vector.tensor_tensor(aw[:], a01[:], a23[:], Alu.max)
    acc = const.tile([128, 256], f32)
    nc.vector.tensor_tensor(acc[:], aw[:, 0:256], aw[:, 256:512], Alu.max)

    ps_cv = ps_pool.tile([128, 256], f32, tag="ps")
    nc.tensor.matmul(
        ps_cv[:], ones_1c[:], cvec[:],
        start=True, stop=True,
    )
    msgsT = const.tile([128, 256], f32)
    nc.vector.tensor_tensor(msgsT[:], acc[:], msg_nT[:], Alu.add)
    resT = const.tile([128, 256], f32r)
    nc.vector.tensor_tensor(resT[:], msgsT[:], ps_cv[:], Alu.max)

    # ret_T (o, n)
    ps_ret = ps_pool.tile([128, 256], f32, tag="ps")
    nc.tensor.matmul(
        ps_ret[:], W_sb["Wo1"][:], nodeT[:],
        start=True, stop=False,
    )
    nc.tensor.matmul(
        ps_ret[:], W_sb["Wo2"][:], hidT[:],
        start=False, stop=False,
    )
    nc.tensor.matmul(
        ps_ret[:], W_sb["Wo3"][:], resT[:],
        start=False, stop=False,
    )
    for k, bname in enumerate(["bo1", "bo2", "bo3"]):
        nc.tensor.matmul(
            ps_ret[:],
            B_sb[bname][:],
            ones_row[:],
            start=False,
            stop=(k == 2),
        )
    retT = const.tile([128, 256], f32)
    nc.scalar.copy(retT[:], ps_ret[:])

    ps_out = ps_pool.tile([128, 256], f32, tag="ps")
    for t in range(NT):
        nc.tensor.transpose(
            ps_out[:, t * 128 : (t + 1) * 128],
            retT[:, t * 128 : (t + 1) * 128],
            ident[:],
        )
    out_sb = const.tile([128, 256], f32)
    nc.scalar.copy(out_sb[:], ps_out[:])
    nc.sync.dma_start(
        out.rearrange("(t p) o -> p t o", p=128),
        out_sb[:].rearrange("p (t o) -> p t o", t=NT),
    )


def build_nc(repeat=1, edge_groups=None, loop_n=1):
    """Build the (single-core SPMD) Bass program; returns nc."""
    _ensure_path()
    import concourse.tile as tile
    from concourse import bacc, mybir

    f32 = mybir.dt.float32
    i32 = mybir.dt.int32

    nc = bacc.Bacc(
        "TRN2", target_bir_lowering=False, debug=False, num_devices=NCORES
    )
    n_edge_rows = N if edge_groups is None else edge_groups * GI
    aps = {
        "edge": nc.dram_tensor(
            "edge", [n_edge_rows, E, N], f32, kind="ExternalInput"
        ).ap(),
        "node": nc.dram_tensor("node", [N, D], f32, kind="ExternalInput").ap(),
        "hidden": nc.dram_tensor("hidden", [N, D], f32, kind="ExternalInput").ap(),
        "graph": nc.dram_tensor("graph", [G], f32, kind="ExternalInput").ap(),
        "adj": nc.dram_tensor("adj", [N, N], i32, kind="ExternalInput").ap(),
        "out": nc.dram_tensor("out", [N, OUT], f32, kind="ExternalOutput").ap(),
    }
    for w in _WNAMES:
        aps[w] = nc.dram_tensor(w, [128, 128], f32, kind="ExternalInput").ap()
    for b in _BNAMES:
        aps[b] = nc.dram_tensor(b, [128], f32, kind="ExternalInput").ap()

    with tile.TileContext(nc) as tc:
        if loop_n > 1:
            with tc.For_i(0, loop_n, 1):
                with ExitStack() as ctx:
                    _kernel_body(ctx, tc, aps, rep=0, edge_groups=edge_groups)
        else:
            for rep in range(repeat):
                with ExitStack() as ctx:
                    _kernel_body(ctx, tc, aps, rep=rep, edge_groups=edge_groups)
    nc.compile()
    return nc


def _get_nc():
    if "nc" not in _CACHE:
        _CACHE["nc"] = build_nc()
    return _CACHE["nc"]


def make_in_maps(**inputs):
    """Shard full inputs into per-core input maps (batch-parallel)."""
    in_maps = []
    for c in range(NCORES):
        m = {
            "edge": np.ascontiguousarray(
                np.asarray(inputs["edge_fts"][c], np.float32).transpose(0, 2, 1)
            ),
            "node": np.ascontiguousarray(inputs["node_fts"][c], np.float32),
            "hidden": np.ascontiguousarray(inputs["hidden"][c], np.float32),
            "graph": np.ascontiguousarray(inputs["graph_fts"][c], np.float32),
            "adj": np.ascontiguousarray(inputs["adj_mat"][c], np.int32),
        }
        for w in _WNAMES:
            m[w] = np.ascontiguousarray(inputs[w], np.float32)
        for b in _BNAMES:
            m[b] = np.ascontiguousarray(inputs[b], np.float32)
        in_maps.append(m)
    return in_maps


def kernel(**inputs) -> np.ndarray:
    """Full-input entry point: shards over 8 cores, returns (B, N, OUT)."""
    _ensure_path()
    from concourse import bass_utils

    nc = _get_nc()
    in_maps = make_in_maps(**inputs)
    res = bass_utils.run_bass_kernel_spmd(nc, in_maps, core_ids=list(range(NCORES)))
    outs = [res.results[c]["out"] for c in range(NCORES)]
    return np.stack(outs, axis=0).astype(np.float32)


def kernel_traced(tmpdir=None, **inputs):
    """Like kernel(), but requests an NTFF profile; returns (out, results)."""
    _ensure_path()
    from concourse import bass_utils

    nc = _get_nc()
    in_maps = make_in_maps(**inputs)
    res = bass_utils.run_bass_kernel_spmd(
        nc, in_maps, core_ids=list(range(NCORES)), trace=True, tmpdir=tmpdir
    )
    outs = [res.results[c]["out"] for c in range(NCORES)]
    return np.stack(outs, axis=0).astype(np.float32), res


if __name__ == "__main__":
    rng = np.random.default_rng(0)
    inputs = {
        "node_fts": rng.normal(size=(B, N, D)).astype(np.float32),
        "edge_fts": rng.normal(size=(B, N, N, E)).astype(np.float32),
        "graph_fts": rng.normal(size=(B, G)).astype(np.float32),
        "adj_mat": rng.integers(0, 2, size=(B, N, N)).astype(np.int32),
        "hidden": rng.normal(size=(B, N, D)).astype(np.float32),
    }
    s = 0.02
    for w in _WNAMES:
        inputs[w] = (s * rng.normal(size=(128, 128))).astype(np.float32)
    for b in _BNAMES:
        inputs[b] = np.zeros(128, np.float32)
    out = kernel(**inputs)
    print(out.shape, out.dtype)


# revision 26
# speedup vs baseline: 2.9941x; 1.1344x over previous
"""Trainium2 Bass kernel for the GNN message-passing module.

Reference computation (per batch b):
    msg_n = node @ Wn + bn                      (N, MID)
    msg_h = hidden @ Wh + bh                    (N, MID)
    msg_e = edge @ We + be                      (N, N, MID)
    msg_g = graph @ Wg + bg                     (MID,)
    msgs[i,j,:] = msg_n[j] + msg_h[i] + msg_e[i,j] + msg_g
    out_msgs[j,:] = max_i(msgs[i,j,:] * adj[i,j])
    ret = node @ Wo1 + bo1 + hidden @ Wo2 + bo2 + out_msgs @ Wo3 + bo3

Kernel strategy (data-parallel, one batch per core across 8 cores):
  - Orientation: channels on SBUF partitions, j (receiver) on the free dim.
  - The multiplicative {0,1} adjacency mask is converted to an additive mask
    adjm = (adj-1)*1e30 in {0, -1e30}, folded into the PE accumulation as a
    rank-1 matmul (ones_c (x) adjm_row_i).  A per-j correction vector cvec
    restores the exact max semantics (masked entries contribute 0 to the max,
    all-kept columns must not see the 0 candidate).
  - msg_n is constant in i, so it is pulled out of the max and added once.
  - h_i = msg_h[i] + msg_g + (bn+bh+be+bg) enters through the fused DVE op
    acc = max(acc, psum_i + h_col_i) (scalar_tensor_tensor, one op per i).
  - fp32 data is fed to the PE as float32r (replicated fp32), which streams at
    1 cycle/row for free dims >= 256 while keeping full fp32 precision.
"""

from contextlib import ExitStack

import numpy as np

B, N, D, E, G, MID, OUT = 8, 256, 128, 128, 128, 128, 128
NCORES = 8
BIG = 1.0e30
GI = 16  # edge rows (i values) per DMA group
CH = 16  # staging chunk == one edge group; quad members are (i, i+4, i+8, i+12)
NT = N // 128  # number of 128-row tiles along N

_WNAMES = ["Wn", "Wh", "We", "Wg", "Wo1", "Wo2", "Wo3"]
_BNAMES = ["bn", "bh", "be", "bg", "bo1", "bo2", "bo3"]

_CACHE = {}


def _ensure_path():
    try:
        import concourse.bass  # noqa: F401
    except ImportError:
        import sys

        for p in ("/opt/trn_rl_repo", "/root/.axon_site/_ro/trn_rl_repo"):
            if p not in sys.path:
                sys.path.insert(0, p)
        import concourse.bass  # noqa: F401


def _kernel_body(ctx, tc, aps, rep=0, edge_groups=None):
    import concourse.bass as bass  # noqa: F401
    from concourse import masks, mybir

    nc = tc.nc
    f32 = mybir.dt.float32
    f32r = mybir.dt.float32r
    Alu = mybir.AluOpType

    edge = aps["edge"]
    node = aps["node"]
    hidden = aps["hidden"]
    graph = aps["graph"]
    adj = aps["adj"]
    out = aps["out"]

    const = ctx.enter_context(tc.tile_pool(name="const", bufs=1))
    ps_pool = ctx.enter_context(tc.tile_pool(name="ps", bufs=2, space="PSUM"))
    opool = ctx.enter_context(tc.tile_pool(name="op", bufs=3, space="PSUM"))
    epool = ctx.enter_context(tc.tile_pool(name="edgein", bufs=3))

    # ---- constants -------------------------------------------------------
    ident = const.tile([128, 128], f32)
    masks.make_identity(nc, ident[:])

    ones_f = const.tile([1, 256], f32)
    nc.vector.memset(ones_f[:], 1.0)
    ones_row = const.tile([1, 256], f32r)
    nc.vector.tensor_copy(ones_row[:], ones_f[:])
    ones_1c = const.tile([1, 128], f32r)
    nc.vector.tensor_copy(ones_1c[:], ones_f[:, 0:128])
    ones_11 = const.tile([1, 1], f32r)
    nc.vector.tensor_copy(ones_11[:], ones_f[:, 0:1])
    ones_colf = const.tile([128, 1], f32)
    nc.vector.memset(ones_colf[:], 1.0)
    ones_col = const.tile([128, 1], f32r)
    nc.vector.tensor_copy(ones_col[:], ones_colf[:])

    W_sb = {}
    for w in _WNAMES:
        Wf = const.tile([128, 128], f32, name=f"r{rep}_Wf_{w}", tag=f"Wf_{w}")
        nc.sync.dma_start(Wf[:], aps[w])
        W_sb[w] = const.tile([128, 128], f32r, name=f"r{rep}_W_{w}", tag=f"W_{w}")
        nc.vector.tensor_copy(W_sb[w][:], Wf[:])
    B_sb = {}
    for b in _BNAMES:
        Bf = const.tile([1, 128], f32, name=f"r{rep}_Bf_{b}", tag=f"Bf_{b}")
        nc.sync.dma_start(Bf[:], aps[b].rearrange("(o k) -> o k", o=1))
        B_sb[b] = const.tile([1, 128], f32r, name=f"r{rep}_B_{b}", tag=f"B_{b}")
        nc.vector.tensor_copy(B_sb[b][:], Bf[:])

    graph_colf = const.tile([128, 1], f32)
    nc.sync.dma_start(graph_colf[:], graph.rearrange("(p o) -> p o", o=1))
    graph_col = const.tile([128, 1], f32r)
    nc.vector.tensor_copy(graph_col[:], graph_colf[:])

    node_nat = const.tile([128, NT * 128], f32)
    nc.sync.dma_start(
        node_nat[:].rearrange("p (t d) -> p t d", t=NT),
        node.rearrange("(t p) d -> p t d", p=128),
    )
    hid_nat = const.tile([128, NT * 128], f32)
    nc.sync.dma_start(
        hid_nat[:].rearrange("p (t d) -> p t d", t=NT),
        hidden.rearrange("(t p) d -> p t d", p=128),
    )
    adj_nat = const.tile([128, NT * 256], mybir.dt.int32)
    nc.sync.dma_start(
        adj_nat[:].rearrange("p (t j) -> p t j", t=NT),
        adj.rearrange("(t p) j -> p t j", p=128),
    )

    # ---- per-batch precompute -------------------------------------------
    # nodeT / hidT: (d, n) layouts via PE transpose
    nodeT = const.tile([128, 256], f32r)
    hidT = const.tile([128, 256], f32r)
    for nat, T in ((node_nat, nodeT), (hid_nat, hidT)):
        ps = ps_pool.tile([128, 256], f32, tag="ps")
        for t in range(NT):
            nc.tensor.transpose(
                ps[:, t * 128 : (t + 1) * 128],
                nat[:, t * 128 : (t + 1) * 128],
                ident[:],
            )
        nc.scalar.copy(T[:], ps[:])

    # r0 = graph @ Wg + (bn + bh + be + bg), a (1, MID) row
    ps_r0 = ps_pool.tile([128, 256], f32, tag="ps")
    nc.tensor.matmul(
        ps_r0[0:1, 0:128],
        graph_col[:],
        W_sb["Wg"][:],
        start=True,
        stop=False,
    )
    for k, bname in enumerate(["bn", "bh", "be", "bg"]):
        nc.tensor.matmul(
            ps_r0[0:1, 0:128],
            ones_11[:],
            B_sb[bname][:],
            start=False,
            stop=(k == 3),
        )
    r0 = const.tile([1, 128], f32r)
    nc.scalar.copy(r0[:], ps_r0[0:1, 0:128])

    # H_nat[i, c] = hidden @ Wh + r0  (h_i rows, natural orientation, f32r)
    ps_h = ps_pool.tile([128, 256], f32, tag="ps")
    for t in range(NT):
        nc.tensor.matmul(
            ps_h[:, t * 128 : (t + 1) * 128],
            hidT[:, t * 128 : (t + 1) * 128],
            W_sb["Wh"][:],
            start=True,
            stop=False,
        )
        nc.tensor.matmul(
            ps_h[:, t * 128 : (t + 1) * 128],
            ones_1c[:],
            r0[:],
            start=False,
            stop=True,
        )
    H_natr = const.tile([128, 256], f32r)
    nc.scalar.copy(H_natr[:], ps_h[:])

    # DRAM scratch for gather-staged reads (adjm writeback emitted later)
    h_dram = aps["h_scratch"]
    a_dram = aps["a_scratch"]
    nc.sync.dma_start(
        h_dram.rearrange("(t p) c -> p t c", p=128),
        H_natr[:].rearrange("p (t c) -> p t c", t=NT),
    )

    # Persistent double-buffered staging tiles for the fused K=3 matmul.
    # Group of GI=16 rows; quad q has members i0+q+8h+4u (h,u in {0,1}).
    # Hab rows: [h_mem0; h_mem1; ones] per (q,h) block of 128.
    # AR3 rows: [sel0; sel1; adjm(mem0|mem1)] per (q,h) block of 512.
    NQ = 4                    # quads per group
    NB = NQ * 2               # (q, h) blocks per group
    habA = const.tile([3, NB * 128], f32r)
    habB = const.tile([3, NB * 128], f32r)
    arA = const.tile([3, NB * 512], f32r)
    arB = const.tile([3, NB * 512], f32r)
    sel0f = const.tile([1, NB * 512], f32)
    nc.vector.memset(sel0f[:], 0.0)
    nc.vector.memset(
        sel0f[:].rearrange("o (b u j) -> o b u j", b=NB, u=2)[:, :, 0:1, :], 1.0
    )
    sel1f = const.tile([1, NB * 512], f32)
    nc.vector.memset(sel1f[:], 0.0)
    nc.vector.memset(
        sel1f[:].rearrange("o (b u j) -> o b u j", b=NB, u=2)[:, :, 1:2, :], 1.0
    )
    onesw_f = const.tile([1, NB * 128], f32)
    nc.vector.memset(onesw_f[:], 1.0)
    for dst in (arA, arB):
        nc.gpsimd.dma_start(dst[0:1, :], sel0f[:])
        nc.gpsimd.dma_start(dst[1:2, :], sel1f[:])
    for dst in (habA, habB):
        nc.gpsimd.dma_start(dst[2:3, :], onesw_f[:])

    # msg_nT[c, j] = (node @ Wn).T  (no bias: biases live in r0)
    ps_mn = ps_pool.tile([128, 256], f32, tag="ps")
    nc.tensor.matmul(
        ps_mn[:], W_sb["Wn"][:], nodeT[:],
        start=True, stop=True,
    )
    msg_nT = const.tile([128, 256], f32)
    nc.scalar.copy(msg_nT[:], ps_mn[:])

    # adjm = (adj - 1) * BIG  in {0, -BIG}, natural (i%128, (i//128, j)) layout
    adj_f = const.tile([128, NT * 256], f32)
    nc.vector.tensor_copy(adj_f[:], adj_nat[:])
    adjm = const.tile([128, NT * 256], f32r)
    nc.vector.tensor_scalar(adjm[:], adj_f[:], -1.0, BIG, Alu.add, Alu.mult)
    nc.sync.dma_start(
        a_dram.rearrange("(t p) j -> p t j", p=128),
        adjm[:].rearrange("p (t j) -> p t j", t=NT),
    )

    # cvec[j] = -BIG - max(sum_i adjm[i,j], -BIG)  -> -BIG if column fully
    # kept (k=256), else 0 (the "0 candidate" of the reference max)
    ps_s = ps_pool.tile([128, 256], f32, tag="ps")
    for t in range(NT):
        nc.tensor.matmul(
            ps_s[0:1, :],
            ones_col[:],
            adjm[:, t * 256 : (t + 1) * 256],
            start=(t == 0),
            stop=(t == NT - 1),
        )
    # cvec = -BIG if column fully kept (s == 0), else 0  (threshold form is
    # robust to f32r rounding of the BIG constants)
    cvec = const.tile([1, 256], f32r)
    nc.vector.tensor_scalar(cvec[:], ps_s[0:1, :], -1.0e29, -BIG, Alu.is_ge, Alu.mult)

    # running max accumulators (channels x (pair, j)), round-robin so
    # consecutive DVE ops never self-wait on the previous accumulation
    NACC = 4
    accs = []
    for q in range(NACC):
        a_ = const.tile([128, 1024], f32, name=f"r{rep}_acc{q}", tag=f"acc{q}")
        nc.vector.memset(a_[:], -3.0e38)
        accs.append(a_)

    # ---- main loop over sender rows i -----------------------------------
    # Edge arrives pre-transposed from the host as (i, e, j): tiles load
    # directly in matmul orientation (e on partitions).  Per i-pair:
    # PE: 2 matmuls into one PSUM bank; ACT: per-half bias-add (h_i)
    # evacuation to SBUF; DVE: one wide (128,512) running max.
    edge_r = edge.rearrange("i e j -> e i j")
    NG = N // GI

    def stage_a(g):
        """Load group g; returns the (e, (a, j)) tile."""
        i0 = g * GI
        gsrc = g if edge_groups is None else (g % edge_groups)
        is0 = gsrc * GI
        et = epool.tile([128, GI * 256], f32r, tag="et", name=f"r{rep}_et{g}")
        # alternate the two DMA issue rings so neither sequencer serializes
        dma_eng = nc.sync if g % 2 == 0 else nc.gpsimd
        dma_eng.dma_start(
            et[:].rearrange("p (a j) -> p a j", a=GI),
            edge_r[:, is0 : is0 + GI, :].bitcast(f32r),
        )
        return et

    def stage_b(g, et, chunk):
        """msg_e matmuls + fused (h, adjm) rank-3 matmuls + wide running max."""
        AR3, Hab = chunk
        # et free layout: a = 8h + 4u + q  ->  (h, u, q, j)
        et_r = et[:].rearrange("p (h u q j) -> p h u q j", h=2, u=2, q=NQ)
        for q in range(NQ):
            op = opool.tile([128, 1024], f32, tag="op", name=f"r{rep}_op{g}_{q}")
            for h in range(2):
                b = q * 2 + h
                nc.tensor.matmul(
                    op[:, h * 512 : (h + 1) * 512].rearrange(
                        "p (u j) -> p u j", u=2
                    ),
                    W_sb["We"][:],
                    et_r[:, h, :, q, :],
                    start=True, stop=False,
                )
                nc.tensor.matmul(
                    op[:, h * 512 : (h + 1) * 512],
                    Hab[0:3, b * 128 : (b + 1) * 128],
                    AR3[0:3, b * 512 : (b + 1) * 512],
                    start=False,
                    stop=True,
                )
            a_ = accs[(g * NQ + q) % NACC]
            nc.vector.tensor_tensor(a_[:], op[:], a_[:], Alu.max)

    hv = h_dram.rearrange("(z h u q) c -> z h u q c", h=2, u=2, q=4)
    av = a_dram.rearrange("(z h u q) j -> z h u q j", h=2, u=2, q=4)

    def ar_stage(i0):
        """Stage adjm rows + h rows for group i0 from DRAM (ACT ring)."""
        k = i0 // CH
        AR3, Hab = (arA, habA) if k % 2 == 0 else (arB, habB)
        z = i0 // CH
        # member (q, h, u) -> row i0 + q + 8h + 4u
        for u in range(2):
            nc.scalar.dma_start(
                Hab[u : u + 1, :].rearrange("o (q h c) -> o q h c", q=NQ, h=2),
                hv[z : z + 1, :, u, :, :].transpose([0, 2, 1, 3]),
            )
        nc.scalar.dma_start(
            AR3[2:3, :].rearrange("o (q h u j) -> o q h u j", q=NQ, h=2, u=2),
            av[z : z + 1].transpose([0, 3, 1, 2, 4]),
        )
        return (AR3, Hab)

    prev = None          # (g, et, chunk)
    for g in range(NG):
        ck = ar_stage(g * GI)
        et = stage_a(g)
        if prev is not None:
            stage_b(prev[0], prev[1], prev[2])
        prev = (g, et, ck)
    stage_b(prev[0], prev[1], prev[2])

    # ---- finalize --------------------------------------------------------
    a01 = const.tile([128, 1024], f32)
    nc.vector.tensor_tensor(a01[:], accs[0][:], accs[1][:], Alu.max)
    a23 = const.tile([128, 1024], f32)
    nc.vector.tensor_tensor(a23[:], accs[2][:], accs[3][:], Alu.max)
    aw = const.tile([128, 1024], f32)
    nc.vector.tensor_tensor(aw[:], a01[:], a23[:], Alu.max)
    ah = const.tile([128, 512], f32)
    nc.vector.tensor_tensor(ah[:], aw[:, 0:512], aw[:, 512:1024], Alu.max)
    acc = const.tile([128, 256], f32)
    nc.vector.tensor_tensor(acc[:], ah[:, 0:256], ah[:, 256:512], Alu.max)

    ps_cv = ps_pool.tile([128, 256], f32, tag="ps")
    nc.tensor.matmul(
        ps_cv[:], ones_1c[:], cvec[:],
        start=True, stop=True,
    )
    msgsT = const.tile([128, 256], f32)
    nc.vector.tensor_tensor(msgsT[:], acc[:], msg_nT[:], Alu.add)
    resT = const.tile([128, 256], f32r)
    nc.vector.tensor_tensor(resT[:], msgsT[:], ps_cv[:], Alu.max)

    # ret_T (o, n)
    ps_ret = ps_pool.tile([128, 256], f32, tag="ps")
    nc.tensor.matmul(
        ps_ret[:], W_sb["Wo1"][:], nodeT[:],
        start=True, stop=False,
    )
    nc.tensor.matmul(
        ps_ret[:], W_sb["Wo2"][:], hidT[:],
        start=False, stop=False,
    )
    nc.tensor.matmul(
        ps_ret[:], W_sb["Wo3"][:], resT[:],
        start=False, stop=False,
    )
    for k, bname in enumerate(["bo1", "bo2", "bo3"]):
        nc.tensor.matmul(
            ps_ret[:],
            B_sb[bname][:],
            ones_row[:],
            start=False,
            stop=(k == 2),
        )
    retT = const.tile([128, 256], f32)
    nc.scalar.copy(retT[:], ps_ret[:])

    ps_out = ps_pool.tile([128, 256], f32, tag="ps")
    for t in range(NT):
        nc.tensor.transpose(
            ps_out[:, t * 128 : (t + 1) * 128],
            retT[:, t * 128 : (t + 1) * 128],
            ident[:],
        )
    out_sb = const.tile([128, 256], f32)
    nc.scalar.copy(out_sb[:], ps_out[:])
    nc.sync.dma_start(
        out.rearrange("(t p) o -> p t o", p=128),
        out_sb[:].rearrange("p (t o) -> p t o", t=NT),
    )


def build_nc(repeat=1, edge_groups=None, loop_n=1):
    """Build the (single-core SPMD) Bass program; returns nc."""
    _ensure_path()
    import concourse.tile as tile
    from concourse import bacc, mybir

    f32 = mybir.dt.float32
    i32 = mybir.dt.int32

    nc = bacc.Bacc(
        "TRN2", target_bir_lowering=False, debug=False, num_devices=NCORES
    )
    n_edge_rows = N if edge_groups is None else edge_groups * GI
    aps = {
        "edge": nc.dram_tensor(
            "edge", [n_edge_rows, E, N], f32, kind="ExternalInput"
        ).ap(),
        "node": nc.dram_tensor("node", [N, D], f32, kind="ExternalInput").ap(),
        "hidden": nc.dram_tensor("hidden", [N, D], f32, kind="ExternalInput").ap(),
        "graph": nc.dram_tensor("graph", [G], f32, kind="ExternalInput").ap(),
        "adj": nc.dram_tensor("adj", [N, N], i32, kind="ExternalInput").ap(),
        "out": nc.dram_tensor("out", [N, OUT], f32, kind="ExternalOutput").ap(),
    }
    for w in _WNAMES:
        aps[w] = nc.dram_tensor(w, [128, 128], f32, kind="ExternalInput").ap()
    for b in _BNAMES:
        aps[b] = nc.dram_tensor(b, [128], f32, kind="ExternalInput").ap()
    f32r = mybir.dt.float32r
    aps["h_scratch"] = nc.dram_tensor("h_scratch", [N, MID], f32r).ap()
    aps["a_scratch"] = nc.dram_tensor("a_scratch", [N, N], f32r).ap()

    with tile.TileContext(nc) as tc:
        if loop_n > 1:
            with tc.For_i(0, loop_n, 1):
                with ExitStack() as ctx:
                    _kernel_body(ctx, tc, aps, rep=0, edge_groups=edge_groups)
        else:
            for rep in range(repeat):
                with ExitStack() as ctx:
                    _kernel_body(ctx, tc, aps, rep=rep, edge_groups=edge_groups)
    nc.compile()
    return nc


def _get_nc():
    if "nc" not in _CACHE:
        _CACHE["nc"] = build_nc()
    return _CACHE["nc"]


def make_in_maps(**inputs):
    """Shard full inputs into per-core input maps (batch-parallel)."""
    in_maps = []
    for c in range(NCORES):
        m = {
            "edge": np.ascontiguousarray(
                np.asarray(inputs["edge_fts"][c], np.float32).transpose(0, 2, 1)
            ),
            "node": np.ascontiguousarray(inputs["node_fts"][c], np.float32),
            "hidden": np.ascontiguousarray(inputs["hidden"][c], np.float32),
            "graph": np.ascontiguousarray(inputs["graph_fts"][c], np.float32),
            "adj": np.ascontiguousarray(inputs["adj_mat"][c], np.int32),
        }
        for w in _WNAMES:
            m[w] = np.ascontiguousarray(inputs[w], np.float32)
        for b in _BNAMES:
            m[b] = np.ascontiguousarray(inputs[b], np.float32)
        in_maps.append(m)
    return in_maps


def kernel(**inputs) -> np.ndarray:
    """Full-input entry point: shards over 8 cores, returns (B, N, OUT)."""
    _ensure_path()
    from concourse import bass_utils

    nc = _get_nc()
    in_maps = make_in_maps(**inputs)
    res = bass_utils.run_bass_kernel_spmd(nc, in_maps, core_ids=list(range(NCORES)))
    outs = [res.results[c]["out"] for c in range(NCORES)]
    return np.stack(outs, axis=0).astype(np.float32)


def kernel_traced(tmpdir=None, **inputs):
    """Like kernel(), but requests an NTFF profile; returns (out, results)."""
    _ensure_path()
    from concourse import bass_utils

    nc = _get_nc()
    in_maps = make_in_maps(**inputs)
    res = bass_utils.run_bass_kernel_spmd(
        nc, in_maps, core_ids=list(range(NCORES)), trace=True, tmpdir=tmpdir
    )
    outs = [res.results[c]["out"] for c in range(NCORES)]
    return np.stack(outs, axis=0).astype(np.float32), res


if __name__ == "__main__":
    rng = np.random.default_rng(0)
    inputs = {
        "node_fts": rng.normal(size=(B, N, D)).astype(np.float32),
        "edge_fts": rng.normal(size=(B, N, N, E)).astype(np.float32),
        "graph_fts": rng.normal(size=(B, G)).astype(np.float32),
        "adj_mat": rng.integers(0, 2, size=(B, N, N)).astype(np.int32),
        "hidden": rng.normal(size=(B, N, D)).astype(np.float32),
    }
    s = 0.02
    for w in _WNAMES:
        inputs[w] = (s * rng.normal(size=(128, 128))).astype(np.float32)
    for b in _BNAMES:
        inputs[b] = np.zeros(128, np.float32)
    out = kernel(**inputs)
    print(out.shape, out.dtype)
